# revision 1
# baseline (speedup 1.0000x reference)
"""AdaptiveGCN (2-layer GCNConv + BN eval + adaptive relu/gelu blend) on 8 TRN2 cores.

Strategy (dst-sharded edge-cut partitioning):
  - Nodes sharded across 8 cores by contiguous dst ranges (NL = N/8 per core).
  - Edges (with self-loops) live on the core owning their dst; sorted by
    (dst_block, src_half, dst). dst blocks are 128 nodes wide -> PSUM tiles.
  - Per layer: h = x @ W computed on owner core, scaled by dinv (symmetric-norm
    src factor) and by the BN scale s (folded), AllGathered into a full
    [N, D] gather table. Each core dma_gathers rows h'[src[e]] for its edges,
    multiplies by the one-hot-times-ew matrix M via TensorE matmul
    (contraction over 128 edges), accumulating per-block in PSUM:
        out_block[d, f] = sum_e M[e, d] * h'[src[e], f]
    Then out = psum * dinv[dst] + c (BN fold), adaptive activation blend.
  - deg = segment_sum(ew) computed with the same M tiles against a ones
    column; dinv = 1/sqrt(deg).
  - M ("mew") is built on host from indices + permuted edge weights (a pure
    scatter of input values into a 0/1 structure) and streamed from HBM.

All float compute (matmuls, deg, rsqrt, BN, activations) runs on device.
"""

import dataclasses
import ml_dtypes
import numpy as np
from contextlib import ExitStack

from concourse import bass, bacc, mybir, tile, library_config
from concourse.bass_utils import run_bass_kernel_spmd

F32 = mybir.dt.float32
BF16 = mybir.dt.bfloat16
I16 = mybir.dt.int16
I32 = mybir.dt.int32
AF = mybir.ActivationFunctionType
OP = mybir.AluOpType


@dataclasses.dataclass
class Cfg:
    N: int = 50000
    E: int = 600000
    D: int = 128
    P: int = 8            # cores
    BLK: int = 128        # dst nodes per block
    GM: int = 32          # mew-stream slots per chunk
    bn_eps: float = 1e-5
    gelu_hw: bool = True  # HW Gelu (sim lacks it; False -> Sigmoid stand-in)
    table_bf16: bool = True   # gather table dtype
    mew_bf16: bool = True     # mew stream dtype


# ---------------------------------------------------------------- host prep

def host_prep(x, edge_index, edge_weight, cfg: Cfg):
    """Shard inputs; build per-core index/mew tables and the uniform schedule."""
    N, E, P, BLK = cfg.N, cfg.E, cfg.P, cfg.BLK
    assert x.shape == (N, cfg.D) and cfg.D == 128
    NL = N // P
    assert NL * P == N
    NB = (NL + BLK - 1) // BLK
    NLpad = NB * BLK

    src = np.concatenate([edge_index[0].astype(np.int64), np.arange(N, dtype=np.int64)])
    dst = np.concatenate([edge_index[1].astype(np.int64), np.arange(N, dtype=np.int64)])
    ew = np.concatenate([edge_weight.astype(np.float32), np.ones(N, np.float32)])

    core_of = dst // NL
    # Degree-balanced node->block assignment per core (host-side permutation of
    # each core's local node ids; inverted again at unshard). Equalizes per-block
    # edge counts so the uniform max-over-cores tile counts waste less padding.
    import heapq
    indeg = np.zeros(N, np.int64)
    np.add.at(indeg, dst, 1)
    perms = []     # perms[c][local_old] = local_new (= block*BLK + off)
    for c in range(P):
        deg_c = indeg[c * NL:(c + 1) * NL]
        order_n = np.argsort(-deg_c, kind="stable")
        cap = [BLK] * NB
        cap[NB - 1] = NL - BLK * (NB - 1)
        heap = [(0, bi) for bi in range(NB)]
        heapq.heapify(heap)
        fill = [0] * NB
        pnew = np.zeros(NL, np.int64)
        for ln in order_n:
            while True:
                tot, bi = heapq.heappop(heap)
                if fill[bi] < cap[bi]:
                    break
            pnew[ln] = bi * BLK + fill[bi]
            fill[bi] += 1
            if fill[bi] < cap[bi]:
                heapq.heappush(heap, (tot + int(deg_c[ln]), bi))
        perms.append(pnew)

    HALF = (N + 1) // 2
    assert HALF <= 32767 and (N - HALF) <= 32767
    perm_all = np.concatenate(perms)
    tpos_of = (np.arange(N) // NL) * NL + perm_all  # global node -> table row

    per_core = []
    counts = np.zeros((P, NB, 2), np.int64)
    for c in range(P):
        m = core_of == c
        s, dl, w = tpos_of[src[m]], perms[c][dst[m] - c * NL], ew[m]
        hh = (s >= HALF).astype(np.int64)
        b = dl // BLK
        order = np.lexsort((dl, hh, b))
        s, dl, w, hh, b = s[order], dl[order], w[order], hh[order], b[order]
        per_core.append((s, dl, w, hh, b))
        for bi in range(NB):
            mb_ = b == bi
            counts[c, bi, 0] = np.sum(mb_ & (hh == 0))
            counts[c, bi, 1] = np.sum(mb_ & (hh == 1))

    tiles_bh = np.ceil(counts.max(axis=0) / 128).astype(np.int64)  # [NB,2]
    slots = []          # (b, h, k)
    stream_pos = []
    pos_h = [0, 0]
    for b in range(NB):
        for h in (0, 1):
            for k in range(int(tiles_bh[b, h])):
                slots.append((b, h, k))
                stream_pos.append(pos_h[h])
                pos_h[h] += 1
    T = len(slots)
    T_lo, T_hi = pos_h
    real_edges = counts.sum()
    pad_frac = (T * 128 * P - real_edges) / max(real_edges, 1)

    mew_dt = ml_dtypes.bfloat16 if cfg.mew_bf16 else np.float32
    tab_np = ml_dtypes.bfloat16 if cfg.table_bf16 else np.float32

    in_maps = []
    for c in range(P):
        s, dl, w, hhv, b = per_core[c]
        mew = np.zeros((128, T * 128), np.float32)   # [p, slot*128 + d]
        idx = [np.zeros((16, 8 * max(T_lo, 1)), np.int16),
               np.zeros((16, 8 * max(T_hi, 1)), np.int16)]
        ptr = 0
        for bi in range(NB):
            for hi in (0, 1):
                cnt = int(counts[c, bi, hi])
                es, ed, ewt = s[ptr:ptr + cnt], dl[ptr:ptr + cnt], w[ptr:ptr + cnt]
                ptr += cnt
                ntile = int(tiles_bh[bi, hi])
                base_slot = slots.index((bi, hi, 0)) if ntile else None
                for k in range(ntile):
                    sl = base_slot + k
                    e0 = k * 128
                    ecnt = max(0, min(128, cnt - e0))
                    if ecnt > 0:
                        j = np.arange(ecnt)
                        dd = (ed[e0:e0 + ecnt] - bi * BLK).astype(np.int64)
                        mew[j, sl * 128 + dd] = ewt[e0:e0 + ecnt]
                        sp = stream_pos[sl]
                        ii = (es[e0:e0 + ecnt] - hi * HALF).astype(np.int16)
                        idx[hi][j % 16, sp * 8 + j // 16] = ii
        xT = np.zeros((128, NLpad), np.float32)
        xT[:, perms[c]] = x[c * NL:(c + 1) * NL].T
        in_maps.append({
            "xT": xT,
            "mew": mew.astype(mew_dt),
            "idxlo": np.tile(idx[0], (8, 1)),
            "idxhi": np.tile(idx[1], (8, 1)),
        })

    meta = dict(NL=NL, NB=NB, NLpad=NLpad, T=T, perms=perms, HALF=HALF,
                T_lo=T_lo, T_hi=T_hi, stream_pos=stream_pos,
                slots=slots, tiles_bh=tiles_bh, pad_frac=float(pad_frac))
    return in_maps, meta


def host_consts(W0, b0, W1, b1, gamma0, beta0, mean0, var0,
                gamma1, beta1, mean1, var1, act_params):
    """Replicated (non-sharded) parameter tensors."""
    vecs = np.concatenate([b0, gamma0, beta0, mean0, var0,
                           b1, gamma1, beta1, mean1, var1]).astype(np.float32).reshape(1, 1280)
    ident = np.eye(128, dtype=np.float32)
    return {
        "w0": W0.astype(np.float32),
        "w1": W1.astype(np.float32),
        "vecs": vecs,
        "actp": act_params.reshape(1, 2).astype(np.float32),
        "ident": ident,
    }


# ---------------------------------------------------------------- builder

def build(meta, cfg: Cfg):
    NL, NB, NLpad = meta["NL"], meta["NB"], meta["NLpad"]
    T, HALF = meta["T"], meta["HALF"]
    T_lo, T_hi = meta["T_lo"], meta["T_hi"]
    stream_pos = meta["stream_pos"]
    slots = meta["slots"]
    N, P, GM = cfg.N, cfg.P, cfg.GM
    TDT = BF16 if cfg.table_bf16 else F32
    MDT = BF16 if cfg.mew_bf16 else F32
    gelu_fn = AF.Gelu if cfg.gelu_hw else AF.Sigmoid

    nc = bacc.Bacc(None, target_bir_lowering=False, debug=False)

    xT_ext = nc.declare_dram_parameter("xT", [128, NLpad], F32, isOutput=False)
    mew_ext = nc.declare_dram_parameter("mew", [128, T * 128], MDT, isOutput=False)
    idxlo_ext = nc.declare_dram_parameter("idxlo", [128, 8 * max(T_lo, 1)], I16, isOutput=False)
    idxhi_ext = nc.declare_dram_parameter("idxhi", [128, 8 * max(T_hi, 1)], I16, isOutput=False)
    w0_ext = nc.declare_dram_parameter("w0", [128, 128], F32, isOutput=False)
    w1_ext = nc.declare_dram_parameter("w1", [128, 128], F32, isOutput=False)
    vecs_ext = nc.declare_dram_parameter("vecs", [1, 1280], F32, isOutput=False)
    actp_ext = nc.declare_dram_parameter("actp", [1, 2], F32, isOutput=False)
    ident_ext = nc.declare_dram_parameter("ident", [128, 128], F32, isOutput=False)
    out_ext = nc.declare_dram_parameter("out", [NL, 128], F32, isOutput=True)

    hs_loc = nc.dram_tensor("hs_loc", [NL, 128], TDT)
    hs_full = nc.dram_tensor("hs_full", [N, 128], TDT, addr_space="Shared")
    hs2_loc = nc.dram_tensor("hs2_loc", [NL, 128], TDT)
    hs2_full = nc.dram_tensor("hs2_full", [N, 128], TDT, addr_space="Shared")

    groups = [list(range(P))]

    # chunk layout for the mew stream (slot-stream order)
    mew_chunk_of = [s // GM for s in range(T)]

    with tile.TileContext(nc, num_cores=P) as tc, ExitStack() as ctx:
        nc.gpsimd.load_library(library_config.mlp)
        cst = ctx.enter_context(tc.tile_pool(name="cst", bufs=1))
        w0_sb = cst.tile([128, 128], F32)
        w1_sb = cst.tile([128, 128], F32)
        vecs_sb = cst.tile([1, 1280], F32)
        actp_sb = cst.tile([1, 2], F32)
        ident_sb = cst.tile([128, 128], F32)
        ones_col = cst.tile([128, 1], MDT)
        ones_row = cst.tile([1, 128], F32)
        idxlo_sb = cst.tile([128, 8 * max(T_lo, 1)], I16)
        idxhi_sb = cst.tile([128, 8 * max(T_hi, 1)], I16)
        deg_sb = cst.tile([128, NB], F32)
        dinv_sb = cst.tile([128, NB], F32)
        alpha_col = cst.tile([128, 1], F32)
        nalpha_col = cst.tile([128, 1], F32)
        s0_rep = cst.tile([128, 128], F32)   # BN scale row replicated
        s1_rep = cst.tile([128, 128], F32)
        c0_rep = cst.tile([128, 128], F32)   # BN offset row replicated
        c1_rep = cst.tile([128, 128], F32)
        y1_region = cst.tile([128, NB * 128], F32)
        hs_region = cst.tile([128, NB * 128], TDT)
        hs2_region = cst.tile([128, NB * 128], TDT)
        scratch = cst.tile([1, 6 * 128], F32)  # cols: s0,c0,s1,c1,tmp,alpha

        nc.sync.dma_start(out=w0_sb[:, :], in_=w0_ext[:, :])
        nc.sync.dma_start(out=w1_sb[:, :], in_=w1_ext[:, :])
        nc.sync.dma_start(out=vecs_sb[:, :], in_=vecs_ext[:, :])
        nc.sync.dma_start(out=actp_sb[:, :], in_=actp_ext[:, :])
        nc.sync.dma_start(out=ident_sb[:, :], in_=ident_ext[:, :])
        nc.sync.dma_start(out=idxlo_sb[:, :], in_=idxlo_ext[:, :])
        nc.sync.dma_start(out=idxhi_sb[:, :], in_=idxhi_ext[:, :])
        nc.vector.memset(ones_col[:, :], 1.0)
        nc.vector.memset(ones_row[:, :], 1.0)

        # ---------------- pass 0: deg via mew @ ones
        mew_tiles = {}

        def mew_chunk(sl, pool):
            ch = mew_chunk_of[sl]
            if ch not in mew_tiles:
                lo = ch * GM
                hi = min(T, lo + GM)
                t_ = pool.tile([128, (hi - lo) * 128], MDT, tag="mewc")
                nc.sync.dma_start(out=t_[:, :], in_=mew_ext[:, lo * 128:hi * 128])
                mew_tiles.clear()
                mew_tiles[ch] = (t_, lo)
            t_, lo = mew_tiles[ch]
            return t_[:, (sl - lo) * 128:(sl - lo + 1) * 128]

        shared_mpool = ctx.enter_context(tc.tile_pool(name="sh_mew", bufs=3))
        shared_gpool = ctx.enter_context(tc.tile_pool(name="sh_g", bufs=3))
        shared_psm = ctx.enter_context(tc.tile_pool(name="sh_ps", bufs=4, space="PSUM"))
        shared_wk = ctx.enter_context(tc.tile_pool(name="sh_wk", bufs=3))
        shared_psh = ctx.enter_context(tc.tile_pool(name="sh_psh", bufs=1, space="PSUM"))
        shared_pst = ctx.enter_context(tc.tile_pool(name="sh_pst", bufs=1, space="PSUM"))
        shared_sbp = ctx.enter_context(tc.tile_pool(name="sh_sbp", bufs=3))
        psd = ctx.enter_context(tc.tile_pool(name="sh_psd", bufs=2, space="PSUM"))
        si = 0
        for b in range(NB):
            nsl = sum(1 for s_ in slots if s_[0] == b)
            pd = psd.tile([128, 1], F32)
            for j in range(nsl):
                m_ap = mew_chunk(si + j, shared_mpool)
                nc.tensor.matmul(pd[:, :], m_ap, ones_col[:, :],
                                 start=(j == 0), stop=(j == nsl - 1))
            si += nsl
            nc.scalar.activation(deg_sb[:, b:b + 1], pd[:, :], AF.Copy)
        mew_tiles.clear()

        # ---------------- scalar prep
        # dinv = 1/max(sqrt(deg), .5)  (deg>=1 for real nodes; pads land on 2.0)
        nc.scalar.activation(dinv_sb[:, :], deg_sb[:, :], AF.Sqrt)
        nc.vector.tensor_scalar_max(dinv_sb[:, :], dinv_sb[:, :], 0.5)
        nc.vector.reciprocal(dinv_sb[:, :], dinv_sb[:, :])

        # BN folds: s = gamma / sqrt(var+eps); c = (b - mean) * s + beta
        def vrow(i):
            return vecs_sb[0:1, i * 128:(i + 1) * 128]
        s0 = scratch[0:1, 0:128]; c0 = scratch[0:1, 128:256]
        s1 = scratch[0:1, 256:384]; c1 = scratch[0:1, 384:512]
        tmp = scratch[0:1, 512:640]
        nc.vector.tensor_scalar_add(tmp, vrow(4), cfg.bn_eps)
        nc.scalar.activation(s0, tmp, AF.Sqrt)
        nc.vector.reciprocal(s0, s0)
        nc.vector.tensor_mul(s0, s0, vrow(1))
        nc.vector.tensor_sub(tmp, vrow(0), vrow(3))
        nc.vector.tensor_mul(tmp, tmp, s0)
        nc.vector.tensor_add(c0, tmp, vrow(2))
        nc.vector.tensor_scalar_add(tmp, vrow(9), cfg.bn_eps)
        nc.scalar.activation(s1, tmp, AF.Sqrt)
        nc.vector.reciprocal(s1, s1)
        nc.vector.tensor_mul(s1, s1, vrow(6))
        nc.vector.tensor_sub(tmp, vrow(5), vrow(8))
        nc.vector.tensor_mul(tmp, tmp, s1)
        nc.vector.tensor_add(c1, tmp, vrow(7))

        # alpha = sigmoid(actp[0]); broadcast alpha and rows via K=1 matmuls
        alpha11 = scratch[0:1, 640:641]
        nc.scalar.activation(alpha11, actp_sb[0:1, 0:1], AF.Sigmoid)
        for row, rep in ((s0, s0_rep), (c0, c0_rep), (s1, s1_rep), (c1, c1_rep)):
            pr = shared_psh.tile([128, 128], F32, tag="h")
            nc.tensor.matmul(pr[:, :], ones_row[:, :], row)
            nc.scalar.activation(rep[:, :], pr[:, :], AF.Copy)
        pa = shared_psh.tile([128, 1], F32, tag="h")
        nc.tensor.matmul(pa[:, :], ones_row[:, :], alpha11)
        nc.scalar.activation(alpha_col[:, :], pa[:, :], AF.Copy)
        # 1 - alpha
        nc.vector.tensor_scalar(nalpha_col[:, :], alpha_col[:, :], -1.0, 1.0,
                                OP.mult, OP.add)

        # ---------------- layer matmul + table build helper
        def build_table(src_region, w_sb, s_rep, dst_region, transpose_first):
            """dst_region[:, t*128:+128] = ((src^T) @ W) * dinv_col * s_row."""
            psh, pst, sbp = shared_psh, shared_pst, shared_sbp
            for t in range(NB):
                col = slice(t * 128, (t + 1) * 128)
                if transpose_first:
                    ptr_ = pst.tile([128, 128], F32, tag="tr")
                    nc.tensor.transpose(ptr_[:, :], src_region[:, col], ident_sb[:, :])
                    lhsT = sbp.tile([128, 128], F32, tag="lhs")
                    nc.vector.tensor_copy(lhsT[:, :], ptr_[:, :])
                else:
                    lhsT = sbp.tile([128, 128], F32, tag="lhs")
                    nc.sync.dma_start(out=lhsT[:, :], in_=xT_ext[:, col])
                ph = psh.tile([128, 128], F32, tag="h")
                nc.tensor.matmul(ph[:, :], lhsT[:, :], w_sb[:, :])
                u = sbp.tile([128, 128], F32, tag="u")
                nc.vector.tensor_scalar(u[:, :], ph[:, :], dinv_sb[:, t:t + 1], None,
                                        OP.mult)
                nc.vector.tensor_mul(dst_region[:, col], u[:, :], s_rep[:, :])

        def _store_region(region, loc):
            full_nb = NL // 128
            rem = NL - full_nb * 128
            if full_nb:
                nc.sync.dma_start(
                    out=loc[0:full_nb * 128, :].rearrange("(b p) f -> p b f", p=128),
                    in_=region[:, 0:full_nb * 128].rearrange("p (b f) -> p b f", f=128))
            if rem:
                nc.sync.dma_start(
                    out=loc[full_nb * 128:NL, :],
                    in_=region[0:rem, full_nb * 128:(full_nb + 1) * 128])

        # ---------------- scatter pass helper
        def scatter_pass(table_full, post_fn):
            """Per block: psum += M_slot^T @ swdge-gathered slot; post_fn(b, psum)."""
            mpool, gpool, psm, wk = shared_mpool, shared_gpool, shared_psm, shared_wk
            GS = 8  # slots per dma_gather (1024-idx ucode cap)
            g_tiles = [{}, {}]
            idx_sb = [idxlo_sb, idxhi_sb]
            half_view = [table_full[0:HALF, :], table_full[HALF:N, :]]
            T_h = [T_lo, T_hi]

            def g_slot(h, pos):
                ch = pos // GS
                if ch not in g_tiles[h]:
                    lo = ch * GS
                    hi = min(T_h[h], lo + GS)
                    S = hi - lo
                    t_ = gpool.tile([128, S, 128], TDT, tag=f"gt{h}")
                    nc.gpsimd.dma_gather(
                        t_[:, :, :], half_view[h], idx_sb[h][:, lo * 8:hi * 8],
                        num_idxs=S * 128, num_idxs_reg=S * 128, elem_size=128)
                    g_tiles[h].clear()
                    g_tiles[h][ch] = (t_, lo)
                t_, lo = g_tiles[h][ch]
                return t_[:, pos - lo, :]

            si = 0
            for b in range(NB):
                nsl = sum(1 for s_ in slots if s_[0] == b)
                pm = psm.tile([128, 128], F32, tag="pm")
                for j in range(nsl):
                    sl = si + j
                    _, h, _ = slots[sl]
                    m_ap = mew_chunk(sl, mpool)
                    g_ap = g_slot(h, stream_pos[sl])
                    nc.tensor.matmul(pm[:, :], m_ap, g_ap,
                                     start=(j == 0), stop=(j == nsl - 1))
                si += nsl
                post_fn(b, pm, wk)
            mew_tiles.clear()

        # ---------------- layer 1
        build_table(None, w0_sb, s0_rep, hs_region, transpose_first=False)
        _store_region(hs_region, hs_loc)
        nc.gpsimd.collective_compute(
            "AllGather", OP.bypass, replica_groups=groups,
            ins=[hs_loc[:, :]], outs=[hs_full[:, :]])

        def post1(b, pm, wk):
            col = slice(b * 128, (b + 1) * 128)
            u = wk.tile([128, 128], F32, tag="u")
            nc.vector.tensor_scalar(u[:, :], pm[:, :], dinv_sb[:, b:b + 1], None, OP.mult)
            nc.vector.tensor_add(u[:, :], u[:, :], c0_rep[:, :])
            r = wk.tile([128, 128], F32, tag="r")
            g = wk.tile([128, 128], F32, tag="g")
            nc.scalar.activation(r[:, :], u[:, :], AF.Relu)
            nc.scalar.activation(g[:, :], u[:, :], gelu_fn)
            nc.vector.tensor_scalar(r[:, :], r[:, :], alpha_col[:, 0:1], None, OP.mult)
            nc.vector.tensor_scalar(g[:, :], g[:, :], nalpha_col[:, 0:1], None, OP.mult)
            nc.vector.tensor_add(y1_region[:, col], r[:, :], g[:, :])

        scatter_pass(hs_full, post1)

        # ---------------- layer 2
        build_table(y1_region, w1_sb, s1_rep, hs2_region, transpose_first=True)
        _store_region(hs2_region, hs2_loc)
        nc.gpsimd.collective_compute(
            "AllGather", OP.bypass, replica_groups=groups,
            ins=[hs2_loc[:, :]], outs=[hs2_full[:, :]])

        out_region = y1_region  # reuse (y1 dead after build_table)

        def post2(b, pm, wk):
            col = slice(b * 128, (b + 1) * 128)
            u = wk.tile([128, 128], F32, tag="u")
            nc.vector.tensor_scalar(u[:, :], pm[:, :], dinv_sb[:, b:b + 1], None, OP.mult)
            nc.vector.tensor_add(out_region[:, col], u[:, :], c1_rep[:, :])

        scatter_pass(hs2_full, post2)

        # store out
        full_nb = NL // 128
        rem = NL - full_nb * 128
        if full_nb:
            nc.sync.dma_start(
                out=out_ext[0:full_nb * 128, :].rearrange("(b p) f -> p b f", p=128),
                in_=out_region[:, 0:full_nb * 128].rearrange("p (b f) -> p b f", f=128))
        if rem:
            nc.sync.dma_start(
                out=out_ext[full_nb * 128:NL, :],
                in_=out_region[0:rem, full_nb * 128:(full_nb + 1) * 128])

    nc.finalize()
    return nc


# ---------------------------------------------------------------- runners

def prep_all(inputs, cfg: Cfg):
    in_maps, meta = host_prep(inputs["x"], inputs["edge_index"],
                              inputs["edge_weight"], cfg)
    consts = host_consts(inputs["W0"], inputs["b0"], inputs["W1"], inputs["b1"],
                         inputs["gamma0"], inputs["beta0"], inputs["mean0"],
                         inputs["var0"], inputs["gamma1"], inputs["beta1"],
                         inputs["mean1"], inputs["var1"], inputs["act_params"])
    for m in in_maps:
        m.update(consts)
    return in_maps, meta


def unshard(results, cfg: Cfg, meta=None):
    NL = cfg.N // cfg.P
    out = np.zeros((cfg.N, cfg.D), np.float32)
    for c in range(cfg.P):
        r = results[c]["out"]
        if meta is not None and "perms" in meta:
            out[c * NL:(c + 1) * NL] = r[meta["perms"][c]]
        else:
            out[c * NL:(c + 1) * NL] = r
    return out


# ---------------------------------------------------------------- entrypoint

def _install_dge_patch():
    """walrus needs --dge-levels=vector_dynamic_offsets for the indirect
    (DynamicAP) gather DMAs this kernel uses."""
    from concourse import bass_utils as _bu
    if getattr(_bu, "_gcn_dge_patched", False):
        return
    _orig = _bu.run_command

    def _patched(argv, **kwargs):
        if argv and "walrus_driver" in str(argv[0]) and not any(
                str(a).startswith("--dge-levels") for a in argv):
            argv = list(argv) + ["--dge-levels=vector_dynamic_offsets"]
        return _orig(argv, **kwargs)

    _bu.run_command = _patched
    _bu._gcn_dge_patched = True


_CFG = Cfg()


def kernel(**inputs):
    """Full-input entrypoint: shard, run on 8 NeuronCores, gather output."""
    import numpy as np
    _install_dge_patch()
    inputs = {k: np.asarray(v) for k, v in inputs.items()}
    in_maps, meta = prep_all(inputs, _CFG)
    nc = build(meta, _CFG)
    res = run_bass_kernel_spmd(nc, in_maps, core_ids=list(range(_CFG.P)))
    return unshard([{k: np.asarray(v) for k, v in r.items()} for r in res.results],
                   _CFG, meta)



# revision 2
# speedup vs baseline: 1.0080x; 1.0080x over previous
"""AdaptiveGCN v10 (src-sorted windows): packed gather windows (no per-block 128-ceil).

Edges per core sorted by (half, block, dst). Per (block, half) padded to the
max count over cores (uniform schedule), then the padded stream is chopped
into 128-row gather windows IGNORING block boundaries. A window crossing a
block boundary is consumed by one matmul per touched block ("segment"), with
the one-hot M zero outside the segment's row range (host zeroes ew there).
Gather calls = windows/8 per half -> ~12% fewer Pool-serialized calls than
per-block slot padding, and gather DMA bytes drop by the pad delta.

Also: deg via per-dst-row ew reduce; self-loops folded as local table add;
BN scale folded into W on host; layer-2 table built inside post1.
"""

import dataclasses
import ml_dtypes
import numpy as np
from contextlib import ExitStack

from concourse import bass, bacc, mybir, tile, library_config
from concourse.bass_utils import run_bass_kernel_spmd

F32 = mybir.dt.float32
BF16 = mybir.dt.bfloat16
I32 = mybir.dt.int32
I16 = mybir.dt.int16
AF = mybir.ActivationFunctionType
OP = mybir.AluOpType


@dataclasses.dataclass
class Cfg:
    N: int = 50000
    E: int = 600000
    D: int = 128
    P: int = 8
    BLK: int = 128
    GS: int = 8           # windows per dma_gather call
    bn_eps: float = 1e-5
    gelu_hw: bool = True


# ---------------------------------------------------------------- host prep

def host_prep(x, edge_index, edge_weight, cfg: Cfg):
    N, E, P, BLK = cfg.N, cfg.E, cfg.P, cfg.BLK
    assert x.shape == (N, cfg.D) and cfg.D == 128
    NL = N // P
    NB = (NL + BLK - 1) // BLK
    NLpad = NB * BLK

    src = edge_index[0].astype(np.int64)
    dst = edge_index[1].astype(np.int64)
    ew = edge_weight.astype(np.float32)

    core_of = dst // NL
    import heapq
    indeg = np.zeros(N, np.int64)
    np.add.at(indeg, dst, 1)
    perms = []
    for c in range(P):
        deg_c = indeg[c * NL:(c + 1) * NL]
        order_n = np.argsort(-deg_c, kind="stable")
        cap = [BLK] * NB
        cap[NB - 1] = NL - BLK * (NB - 1)
        heap = [(0, bi) for bi in range(NB)]
        heapq.heapify(heap)
        fill = [0] * NB
        pnew = np.zeros(NL, np.int64)
        for ln in order_n:
            while True:
                tot, bi = heapq.heappop(heap)
                if fill[bi] < cap[bi]:
                    break
            pnew[ln] = bi * BLK + fill[bi]
            fill[bi] += 1
            if fill[bi] < cap[bi]:
                heapq.heappush(heap, (tot + int(deg_c[ln]), bi))
        perms.append(pnew)

    perm_all = np.concatenate(perms)
    tpos_of = (np.arange(N) // NL) * NL + perm_all

    HALF = (N + 1) // 2
    assert HALF <= 32767 and (N - HALF) <= 32767

    per_core = []
    counts = np.zeros((P, NB, 2), np.int64)
    K = 1
    for c in range(P):
        m = core_of == c
        s, dl, w = tpos_of[src[m]], perms[c][dst[m] - c * NL], ew[m]
        hh = (s >= HALF).astype(np.int64)
        b = dl // BLK
        order = np.lexsort((dl, b, hh))   # half outermost
        s, dl, w, hh, b = s[order], dl[order], w[order], hh[order], b[order]
        per_core.append((s, dl, w, hh, b))
        for bi in range(NB):
            mb_ = b == bi
            counts[c, bi, 0] = np.sum(mb_ & (hh == 0))
            counts[c, bi, 1] = np.sum(mb_ & (hh == 1))
        cnt_n = np.bincount(dl, minlength=NLpad)
        K = max(K, int(cnt_n.max()))

    cmax = counts.max(axis=0)          # [NB, 2] uniform padded counts
    # windows per half; stream order within half h: blocks 0..NB-1
    W_h = []
    pref = np.zeros((2, NB + 1), np.int64)
    for h in (0, 1):
        pref[h, 1:] = np.cumsum(cmax[:, h])
        W_h.append(int(-(-pref[h, NB] // 128)))
    W_lo, W_hi = W_h
    Wtot = W_lo + W_hi

    # segments (matmul units): per half, per block, windows it spans
    segs = []   # (b, h, w, r0, r1)  rows [r0, r1) within window w (half-local)
    for h in (0, 1):
        for b in range(NB):
            p0, p1 = int(pref[h, b]), int(pref[h, b + 1])
            if p1 == p0:
                continue
            wlo, whi = p0 // 128, (p1 - 1) // 128
            for w in range(wlo, whi + 1):
                r0 = max(0, p0 - w * 128)
                r1 = min(128, p1 - w * 128)
                segs.append((b, h, w, r0, r1))
    S = len(segs)
    real_edges = counts.sum()
    pad_frac = (Wtot * 128 * P - real_edges) / max(real_edges, 1)

    in_maps = []
    for c in range(P):
        s, dl, w, hh, b = per_core[c]
        ewrow = np.zeros((128, NB * K), np.float32)
        fill_n = np.zeros(NLpad, np.int64)
        for e in range(len(dl)):
            n = dl[e]
            ewrow[n % 128, (n // 128) * K + fill_n[n]] = w[e]
            fill_n[n] += 1

        idx = [np.zeros((16, 8 * max(W_lo, 1)), np.int16),
               np.zeros((16, 8 * max(W_hi, 1)), np.int16)]
        dstoff = np.zeros((128, S), np.float32)
        ewc = np.zeros((128, S), np.float32)

        # place this core's edges at padded-stream positions, then re-sort
        # each 128-row window by src table row (HBM locality for the gather;
        # the one-hot M absorbs any within-window permutation)
        ptr = 0
        edge_pos = {}   # (h, stream_pos) -> edge index; only real edges
        for h in (0, 1):
            for bi in range(NB):
                cnt = int(counts[c, bi, h])
                base = int(pref[h, bi])
                for k in range(cnt):
                    edge_pos[(h, base + k)] = ptr + k
                ptr += cnt
        for h in (0, 1):
            for wdx in range(W_h[h]):
                rows = [(h, wdx * 128 + j) for j in range(128)]
                es = [edge_pos.get(r) for r in rows]
                keyed = sorted((e for e in es if e is not None), key=lambda e: s[e])
                npad = sum(1 for e in es if e is None)
                for j, r in enumerate(rows):
                    if j < len(keyed):
                        edge_pos[r] = keyed[j]
                    elif r in edge_pos:
                        del edge_pos[r]
        # idx arrays per window
        for h in (0, 1):
            for wdx in range(W_h[h]):
                for j in range(128):
                    e = edge_pos.get((h, wdx * 128 + j))
                    ii = 0 if e is None else int(s[e] - h * HALF)
                    idx[h][j % 16, wdx * 8 + j // 16] = ii
        # per-seg columns by block membership (rows are permuted within
        # windows, so an edge can sit outside its block's contiguous range)
        seg_of = {(bi, h, wdx): si for si, (bi, h, wdx, r0, r1) in enumerate(segs)}
        for h in (0, 1):
            for wdx in range(W_h[h]):
                for j in range(128):
                    e = edge_pos.get((h, wdx * 128 + j))
                    if e is not None:
                        bb = int(dl[e]) // BLK
                        si = seg_of[(bb, h, wdx)]
                        dstoff[j, si] = float(dl[e] - bb * BLK)
                        ewc[j, si] = w[e]

        xT = np.zeros((128, NLpad), np.float32)
        xT[:, perms[c]] = x[c * NL:(c + 1) * NL].T
        in_maps.append({
            "xT": xT,
            "idxlo": np.tile(idx[0], (8, 1)),
            "idxhi": np.tile(idx[1], (8, 1)),
            "dstoff": dstoff,
            "ewc": ewc,
            "ewrow": ewrow,
        })

    meta = dict(NL=NL, NB=NB, NLpad=NLpad, K=K, HALF=HALF,
                W_lo=W_lo, W_hi=W_hi, S=S, segs=segs, perms=perms,
                pad_frac=float(pad_frac))
    return in_maps, meta


def host_consts(W0, b0, W1, b1, gamma0, beta0, mean0, var0,
                gamma1, beta1, mean1, var1, act_params):
    eps = 1e-5
    s0 = (gamma0 / np.sqrt(var0 + eps)).astype(np.float32)
    s1 = (gamma1 / np.sqrt(var1 + eps)).astype(np.float32)
    c0 = ((b0 - mean0) * s0 + beta0).astype(np.float32)
    c1 = ((b1 - mean1) * s1 + beta1).astype(np.float32)
    crow = np.concatenate([c0, c1]).reshape(1, 256)
    alpha = float(1.0 / (1.0 + np.exp(-float(np.asarray(act_params).reshape(-1)[0]))))
    ident = np.eye(128, dtype=np.float32)
    iota = np.tile(np.arange(128, dtype=np.float32)[None, :], (128, 1)).astype(ml_dtypes.bfloat16)
    return {
        "w0": (W0 * s0[None, :]).astype(np.float32),
        "w1": (W1 * s1[None, :]).astype(np.float32),
        "crow": crow,
        "ident": ident,
        "iota": iota,
    }, alpha


# ---------------------------------------------------------------- builder

def build(meta, cfg: Cfg):
    NL, NB, NLpad = meta["NL"], meta["NB"], meta["NLpad"]
    K, HALF = meta["K"], meta["HALF"]
    W_lo, W_hi, S = meta["W_lo"], meta["W_hi"], meta["S"]
    segs = meta["segs"]
    N, P, GS = cfg.N, cfg.P, cfg.GS
    alpha = float(meta["alpha"])
    gelu_fn = AF.Gelu if cfg.gelu_hw else AF.Sigmoid

    nc = bacc.Bacc(None, target_bir_lowering=False, debug=False)

    xT_ext = nc.declare_dram_parameter("xT", [128, NLpad], F32, isOutput=False)
    idxlo_ext = nc.declare_dram_parameter("idxlo", [128, 8 * max(W_lo, 1)], I16, isOutput=False)
    idxhi_ext = nc.declare_dram_parameter("idxhi", [128, 8 * max(W_hi, 1)], I16, isOutput=False)
    dstoff_ext = nc.declare_dram_parameter("dstoff", [128, S], F32, isOutput=False)
    ewc_ext = nc.declare_dram_parameter("ewc", [128, S], F32, isOutput=False)
    ewrow_ext = nc.declare_dram_parameter("ewrow", [128, NB * K], F32, isOutput=False)
    w0_ext = nc.declare_dram_parameter("w0", [128, 128], F32, isOutput=False)
    w1_ext = nc.declare_dram_parameter("w1", [128, 128], F32, isOutput=False)
    crow_ext = nc.declare_dram_parameter("crow", [1, 256], F32, isOutput=False)
    ident_ext = nc.declare_dram_parameter("ident", [128, 128], F32, isOutput=False)
    iota_ext = nc.declare_dram_parameter("iota", [128, 128], BF16, isOutput=False)
    out_ext = nc.declare_dram_parameter("out", [NL, 128], F32, isOutput=True)

    hs_loc = nc.dram_tensor("hs_loc", [NL, 128], BF16)
    hs_full = nc.dram_tensor("hs_full", [N, 128], BF16, addr_space="Shared")
    hs2_loc = nc.dram_tensor("hs2_loc", [NL, 128], BF16)
    hs2_full = nc.dram_tensor("hs2_full", [N, 128], BF16, addr_space="Shared")

    groups = [list(range(P))]

    with tile.TileContext(nc, num_cores=P) as tc, ExitStack() as ctx:
        nc.gpsimd.load_library(library_config.mlp)
        cst = ctx.enter_context(tc.tile_pool(name="cst", bufs=1))
        w0_sb = cst.tile([128, 128], F32)
        w1_sb = cst.tile([128, 128], F32)
        crow_sb = cst.tile([1, 256], F32)
        ident_sb = cst.tile([128, 128], F32)
        xT_region = cst.tile([128, NLpad], F32)
        iota_sb = cst.tile([128, 128], BF16)
        ones_row = cst.tile([1, 128], F32)
        idxlo_sb = cst.tile([128, 8 * max(W_lo, 1)], I16)
        idxhi_sb = cst.tile([128, 8 * max(W_hi, 1)], I16)
        dstoff_sb = cst.tile([128, S], F32)
        ewc_sb = cst.tile([128, S], F32)
        ewrow_sb = cst.tile([128, NB * K], F32)
        deg_sb = cst.tile([128, NB], F32)
        dinv_sb = cst.tile([128, NB], F32)
        c0_rep = cst.tile([128, 128], F32)
        c1_rep = cst.tile([128, 128], F32)
        y1_region = cst.tile([128, NB * 128], F32)
        hs_region = cst.tile([128, NB * 128], BF16)
        hs2_region = cst.tile([128, NB * 128], BF16)

        nc.sync.dma_start(out=w0_sb[:, :], in_=w0_ext[:, :])
        nc.sync.dma_start(out=w1_sb[:, :], in_=w1_ext[:, :])
        nc.sync.dma_start(out=crow_sb[:, :], in_=crow_ext[:, :])
        nc.sync.dma_start(out=xT_region[:, :], in_=xT_ext[:, :])
        nc.sync.dma_start(out=ident_sb[:, :], in_=ident_ext[:, :])
        nc.sync.dma_start(out=iota_sb[:, :], in_=iota_ext[:, :])
        nc.sync.dma_start(out=idxlo_sb[:, :], in_=idxlo_ext[:, :])
        nc.sync.dma_start(out=idxhi_sb[:, :], in_=idxhi_ext[:, :])
        nc.sync.dma_start(out=dstoff_sb[:, :], in_=dstoff_ext[:, :])
        nc.sync.dma_start(out=ewc_sb[:, :], in_=ewc_ext[:, :])
        nc.sync.dma_start(out=ewrow_sb[:, :], in_=ewrow_ext[:, :])
        nc.vector.memset(ones_row[:, :], 1.0)

        mpool = ctx.enter_context(tc.tile_pool(name="mh", bufs=12))
        gpool = ctx.enter_context(tc.tile_pool(name="gh", bufs=6))
        psm = ctx.enter_context(tc.tile_pool(name="ps", bufs=6, space="PSUM"))
        wk = ctx.enter_context(tc.tile_pool(name="wk", bufs=4))
        psh = ctx.enter_context(tc.tile_pool(name="psh", bufs=1, space="PSUM"))
        pst = ctx.enter_context(tc.tile_pool(name="pst", bufs=1, space="PSUM"))
        sbp = ctx.enter_context(tc.tile_pool(name="sbp", bufs=3))

        def build_m(si, pool):
            m = pool.tile([128, 128], BF16, tag="m")
            nc.vector.tensor_scalar(m[:, :], iota_sb[:, :],
                                    dstoff_sb[:, si:si + 1], ewc_sb[:, si:si + 1],
                                    OP.is_equal, OP.mult)
            return m

        # deg via one reduction of the per-dst-row ew layout; +1 self-loop
        nc.vector.tensor_reduce(
            deg_sb[:, :], ewrow_sb[:, :].rearrange("p (b k) -> p b k", k=K),
            mybir.AxisListType.X, OP.add)
        nc.vector.tensor_scalar_add(deg_sb[:, :], deg_sb[:, :], 1.0)
        nc.scalar.activation(dinv_sb[:, :], deg_sb[:, :], AF.Sqrt)
        nc.vector.tensor_scalar_max(dinv_sb[:, :], dinv_sb[:, :], 0.5)
        nc.vector.reciprocal(dinv_sb[:, :], dinv_sb[:, :])

        for i, rep in ((0, c0_rep), (1, c1_rep)):
            pr = psh.tile([128, 128], F32, tag="h")
            nc.tensor.matmul(pr[:, :], ones_row[:, :], crow_sb[0:1, i * 128:(i + 1) * 128])
            nc.scalar.activation(rep[:, :], pr[:, :], AF.Copy)

        def table_block_l1(t):
            col = slice(t * 128, (t + 1) * 128)
            ph = psh.tile([128, 128], F32, tag="h")
            nc.tensor.matmul(ph[:, :], xT_region[:, col], w0_sb[:, :])
            nc.vector.tensor_scalar(hs_region[:, col], ph[:, :],
                                    dinv_sb[:, t:t + 1], None, OP.mult)

        def table_block_l2(t):
            col = slice(t * 128, (t + 1) * 128)
            ptr_ = pst.tile([128, 128], F32, tag="tr")
            nc.tensor.transpose(ptr_[:, :], y1_region[:, col], ident_sb[:, :])
            lhsT = sbp.tile([128, 128], F32, tag="lhs")
            nc.vector.tensor_copy(lhsT[:, :], ptr_[:, :])
            ph = psh.tile([128, 128], F32, tag="h")
            nc.tensor.matmul(ph[:, :], lhsT[:, :], w1_sb[:, :])
            nc.vector.tensor_scalar(hs2_region[:, col], ph[:, :],
                                    dinv_sb[:, t:t + 1], None, OP.mult)

        def _store_region(region, loc):
            full_nb = NL // 128
            rem = NL - full_nb * 128
            if full_nb:
                nc.sync.dma_start(
                    out=loc[0:full_nb * 128, :].rearrange("(b p) f -> p b f", p=128),
                    in_=region[:, 0:full_nb * 128].rearrange("p (b f) -> p b f", f=128))
            if rem:
                nc.sync.dma_start(
                    out=loc[full_nb * 128:NL, :],
                    in_=region[0:rem, full_nb * 128:(full_nb + 1) * 128])

        # segments grouped by block, in (half, window) order per block
        segs_of_block = [[] for _ in range(NB)]
        for si, (b, h, w, r0, r1) in enumerate(segs):
            segs_of_block[b].append((si, h, w))
        W_half = [W_lo, W_hi]

        def scatter_pass(table_full, post_fn):
            g_tiles = [{}, {}]
            idx_sb = [idxlo_sb, idxhi_sb]
            half_view = [table_full[0:HALF, :], table_full[HALF:N, :]]

            def g_window(h, w):
                ch = w // GS
                if ch not in g_tiles[h]:
                    lo = ch * GS
                    hi = min(W_half[h], lo + GS)
                    Sn = hi - lo
                    t_ = gpool.tile([128, Sn, 128], BF16, tag=f"gt{h}")
                    nc.gpsimd.dma_gather(
                        t_[:, :, :], half_view[h], idx_sb[h][:, lo * 8:hi * 8],
                        num_idxs=Sn * 128, num_idxs_reg=Sn * 128, elem_size=128)
                    g_tiles[h].clear()
                    g_tiles[h][ch] = (t_, lo)
                t_, lo = g_tiles[h][ch]
                return t_[:, w - lo, :]

            for b in range(NB):
                sl = segs_of_block[b]
                pm = psm.tile([128, 128], F32, tag="pm")
                for j, (si, h, w) in enumerate(sl):
                    m = build_m(si, mpool)
                    g_ap = g_window(h, w)
                    nc.tensor.matmul(pm[:, :], m[:, :], g_ap,
                                     start=(j == 0), stop=(j == len(sl) - 1))
                post_fn(b, pm, wk)

        # ---------------- layer 1
        for t in range(NB):
            table_block_l1(t)
        _store_region(hs_region, hs_loc)
        nc.gpsimd.collective_compute(
            "AllGather", OP.bypass, replica_groups=groups,
            ins=[hs_loc[:, :]], outs=[hs_full[:, :]])

        def post1(b, pm, wkp):
            col = slice(b * 128, (b + 1) * 128)
            u = wkp.tile([128, 128], F32, tag="u")
            nc.vector.tensor_add(u[:, :], pm[:, :], hs_region[:, col])
            nc.vector.tensor_scalar(u[:, :], u[:, :], dinv_sb[:, b:b + 1], None, OP.mult)
            nc.vector.tensor_add(u[:, :], u[:, :], c0_rep[:, :])
            r = wkp.tile([128, 128], F32, tag="r")
            g = wkp.tile([128, 128], F32, tag="g")
            nc.scalar.activation(r[:, :], u[:, :], AF.Relu)
            nc.scalar.activation(g[:, :], u[:, :], gelu_fn)
            nc.vector.tensor_scalar(r[:, :], r[:, :], alpha, None, OP.mult)
            nc.vector.tensor_scalar(g[:, :], g[:, :], 1.0 - alpha, None, OP.mult)
            nc.vector.tensor_add(y1_region[:, col], r[:, :], g[:, :])
            table_block_l2(b)

        scatter_pass(hs_full, post1)

        # ---------------- layer 2
        _store_region(hs2_region, hs2_loc)
        nc.gpsimd.collective_compute(
            "AllGather", OP.bypass, replica_groups=groups,
            ins=[hs2_loc[:, :]], outs=[hs2_full[:, :]])

        out_region = y1_region

        def post2(b, pm, wkp):
            col = slice(b * 128, (b + 1) * 128)
            u = wkp.tile([128, 128], F32, tag="u")
            nc.vector.tensor_add(u[:, :], pm[:, :], hs2_region[:, col])
            nc.vector.tensor_scalar(u[:, :], u[:, :], dinv_sb[:, b:b + 1], None, OP.mult)
            nc.vector.tensor_add(out_region[:, col], u[:, :], c1_rep[:, :])

        scatter_pass(hs2_full, post2)

        full_nb = NL // 128
        rem = NL - full_nb * 128
        if full_nb:
            nc.sync.dma_start(
                out=out_ext[0:full_nb * 128, :].rearrange("(b p) f -> p b f", p=128),
                in_=out_region[:, 0:full_nb * 128].rearrange("p (b f) -> p b f", f=128))
        if rem:
            nc.sync.dma_start(
                out=out_ext[full_nb * 128:NL, :],
                in_=out_region[0:rem, full_nb * 128:(full_nb + 1) * 128])

    nc.finalize()
    return nc


# ---------------------------------------------------------------- runners

def prep_all(inputs, cfg: Cfg):
    in_maps, meta = host_prep(inputs["x"], inputs["edge_index"],
                              inputs["edge_weight"], cfg)
    consts, alpha = host_consts(
        inputs["W0"], inputs["b0"], inputs["W1"], inputs["b1"],
        inputs["gamma0"], inputs["beta0"], inputs["mean0"],
        inputs["var0"], inputs["gamma1"], inputs["beta1"],
        inputs["mean1"], inputs["var1"], inputs["act_params"])
    meta["alpha"] = alpha
    for m in in_maps:
        m.update(consts)
    return in_maps, meta


def unshard(results, cfg: Cfg, meta=None):
    NL = cfg.N // cfg.P
    out = np.zeros((cfg.N, cfg.D), np.float32)
    for c in range(cfg.P):
        r = results[c]["out"]
        if meta is not None and "perms" in meta:
            out[c * NL:(c + 1) * NL] = r[meta["perms"][c]]
        else:
            out[c * NL:(c + 1) * NL] = r
    return out


# ---------------------------------------------------------------- entrypoint

def _install_dge_patch():
    """walrus needs --dge-levels=vector_dynamic_offsets for the indirect
    (DynamicAP) gather DMAs this kernel uses."""
    from concourse import bass_utils as _bu
    if getattr(_bu, "_gcn_dge_patched", False):
        return
    _orig = _bu.run_command

    def _patched(argv, **kwargs):
        if argv and "walrus_driver" in str(argv[0]) and not any(
                str(a).startswith("--dge-levels") for a in argv):
            argv = list(argv) + ["--dge-levels=vector_dynamic_offsets"]
        return _orig(argv, **kwargs)

    _bu.run_command = _patched
    _bu._gcn_dge_patched = True


_CFG = Cfg()


def kernel(**inputs):
    """Full-input entrypoint: shard, run on 8 NeuronCores, gather output."""
    import numpy as _np
    _install_dge_patch()
    inputs = {k: _np.asarray(v) for k, v in inputs.items()}
    in_maps, meta = prep_all(inputs, _CFG)
    nc = build(meta, _CFG)
    res = run_bass_kernel_spmd(nc, in_maps, core_ids=list(range(_CFG.P)))
    return unshard([{k: _np.asarray(v) for k, v in r.items()} for r in res.results],
                   _CFG, meta)


# revision 3
# speedup vs baseline: 1.0694x; 1.0609x over previous
"""AdaptiveGCN v11 (streamed out): packed gather windows (no per-block 128-ceil).

Edges per core sorted by (half, block, dst). Per (block, half) padded to the
max count over cores (uniform schedule), then the padded stream is chopped
into 128-row gather windows IGNORING block boundaries. A window crossing a
block boundary is consumed by one matmul per touched block ("segment"), with
the one-hot M zero outside the segment's row range (host zeroes ew there).
Gather calls = windows/8 per half -> ~12% fewer Pool-serialized calls than
per-block slot padding, and gather DMA bytes drop by the pad delta.

Also: deg via per-dst-row ew reduce; self-loops folded as local table add;
BN scale folded into W on host; layer-2 table built inside post1.
"""

import dataclasses
import ml_dtypes
import numpy as np
from contextlib import ExitStack

from concourse import bass, bacc, mybir, tile, library_config
from concourse.bass_utils import run_bass_kernel_spmd

F32 = mybir.dt.float32
BF16 = mybir.dt.bfloat16
I32 = mybir.dt.int32
I16 = mybir.dt.int16
AF = mybir.ActivationFunctionType
OP = mybir.AluOpType


@dataclasses.dataclass
class Cfg:
    N: int = 50000
    E: int = 600000
    D: int = 128
    P: int = 8
    BLK: int = 128
    GS: int = 8           # windows per dma_gather call (1024-idx ucode cap)
    bn_eps: float = 1e-5
    gelu_hw: bool = True


# ---------------------------------------------------------------- host prep

def host_prep(x, edge_index, edge_weight, cfg: Cfg):
    N, E, P, BLK = cfg.N, cfg.E, cfg.P, cfg.BLK
    assert x.shape == (N, cfg.D) and cfg.D == 128
    NL = N // P
    NB = (NL + BLK - 1) // BLK
    NLpad = NB * BLK

    src = edge_index[0].astype(np.int64)
    dst = edge_index[1].astype(np.int64)
    ew = edge_weight.astype(np.float32)

    core_of = dst // NL
    import heapq
    indeg = np.zeros(N, np.int64)
    np.add.at(indeg, dst, 1)
    perms = []
    for c in range(P):
        deg_c = indeg[c * NL:(c + 1) * NL]
        order_n = np.argsort(-deg_c, kind="stable")
        cap = [BLK] * NB
        cap[NB - 1] = NL - BLK * (NB - 1)
        heap = [(0, bi) for bi in range(NB)]
        heapq.heapify(heap)
        fill = [0] * NB
        pnew = np.zeros(NL, np.int64)
        for ln in order_n:
            while True:
                tot, bi = heapq.heappop(heap)
                if fill[bi] < cap[bi]:
                    break
            pnew[ln] = bi * BLK + fill[bi]
            fill[bi] += 1
            if fill[bi] < cap[bi]:
                heapq.heappush(heap, (tot + int(deg_c[ln]), bi))
        perms.append(pnew)

    perm_all = np.concatenate(perms)
    tpos_of = (np.arange(N) // NL) * NL + perm_all

    HALF = (N + 1) // 2
    assert HALF <= 32767 and (N - HALF) <= 32767

    per_core = []
    counts = np.zeros((P, NB, 2), np.int64)
    K = 1
    for c in range(P):
        m = core_of == c
        s, dl, w = tpos_of[src[m]], perms[c][dst[m] - c * NL], ew[m]
        hh = (s >= HALF).astype(np.int64)
        b = dl // BLK
        order = np.lexsort((dl, b, hh))   # half outermost
        s, dl, w, hh, b = s[order], dl[order], w[order], hh[order], b[order]
        per_core.append((s, dl, w, hh, b))
        for bi in range(NB):
            mb_ = b == bi
            counts[c, bi, 0] = np.sum(mb_ & (hh == 0))
            counts[c, bi, 1] = np.sum(mb_ & (hh == 1))
        cnt_n = np.bincount(dl, minlength=NLpad)
        K = max(K, int(cnt_n.max()))

    cmax = counts.max(axis=0)          # [NB, 2] uniform padded counts
    # windows per half; stream order within half h: blocks 0..NB-1
    W_h = []
    pref = np.zeros((2, NB + 1), np.int64)
    for h in (0, 1):
        pref[h, 1:] = np.cumsum(cmax[:, h])
        W_h.append(int(-(-pref[h, NB] // 128)))
    W_lo, W_hi = W_h
    Wtot = W_lo + W_hi

    # segments (matmul units): per half, per block, windows it spans
    segs = []   # (b, h, w, r0, r1)  rows [r0, r1) within window w (half-local)
    for h in (0, 1):
        for b in range(NB):
            p0, p1 = int(pref[h, b]), int(pref[h, b + 1])
            if p1 == p0:
                continue
            wlo, whi = p0 // 128, (p1 - 1) // 128
            for w in range(wlo, whi + 1):
                r0 = max(0, p0 - w * 128)
                r1 = min(128, p1 - w * 128)
                segs.append((b, h, w, r0, r1))
    S = len(segs)
    real_edges = counts.sum()
    pad_frac = (Wtot * 128 * P - real_edges) / max(real_edges, 1)

    in_maps = []
    for c in range(P):
        s, dl, w, hh, b = per_core[c]
        ewrow = np.zeros((128, NB * K), np.float32)
        fill_n = np.zeros(NLpad, np.int64)
        for e in range(len(dl)):
            n = dl[e]
            ewrow[n % 128, (n // 128) * K + fill_n[n]] = w[e]
            fill_n[n] += 1

        idx = [np.zeros((16, 8 * max(W_lo, 1)), np.int16),
               np.zeros((16, 8 * max(W_hi, 1)), np.int16)]
        dstoff = np.zeros((128, S), np.float32)
        ewc = np.zeros((128, S), np.float32)

        # place this core's edges at padded-stream positions, then re-sort
        # each 128-row window by src table row (HBM locality for the gather;
        # the one-hot M absorbs any within-window permutation)
        ptr = 0
        edge_pos = {}   # (h, stream_pos) -> edge index; only real edges
        for h in (0, 1):
            for bi in range(NB):
                cnt = int(counts[c, bi, h])
                base = int(pref[h, bi])
                for k in range(cnt):
                    edge_pos[(h, base + k)] = ptr + k
                ptr += cnt
        for h in (0, 1):
            for wdx in range(W_h[h]):
                rows = [(h, wdx * 128 + j) for j in range(128)]
                es = [edge_pos.get(r) for r in rows]
                keyed = sorted((e for e in es if e is not None), key=lambda e: s[e])
                npad = sum(1 for e in es if e is None)
                for j, r in enumerate(rows):
                    if j < len(keyed):
                        edge_pos[r] = keyed[j]
                    elif r in edge_pos:
                        del edge_pos[r]
        # idx arrays per window
        for h in (0, 1):
            for wdx in range(W_h[h]):
                for j in range(128):
                    e = edge_pos.get((h, wdx * 128 + j))
                    ii = 0 if e is None else int(s[e] - h * HALF)
                    idx[h][j % 16, wdx * 8 + j // 16] = ii
        # per-seg columns by block membership (rows are permuted within
        # windows, so an edge can sit outside its block's contiguous range)
        seg_of = {(bi, h, wdx): si for si, (bi, h, wdx, r0, r1) in enumerate(segs)}
        for h in (0, 1):
            for wdx in range(W_h[h]):
                for j in range(128):
                    e = edge_pos.get((h, wdx * 128 + j))
                    if e is not None:
                        bb = int(dl[e]) // BLK
                        si = seg_of[(bb, h, wdx)]
                        dstoff[j, si] = float(dl[e] - bb * BLK)
                        ewc[j, si] = w[e]

        xT = np.zeros((128, NLpad), np.float32)
        xT[:, perms[c]] = x[c * NL:(c + 1) * NL].T
        in_maps.append({
            "xT": xT,
            "idxlo": np.tile(idx[0], (8, 1)),
            "idxhi": np.tile(idx[1], (8, 1)),
            "dstoff": dstoff,
            "ewc": ewc,
            "ewrow": ewrow,
        })

    meta = dict(NL=NL, NB=NB, NLpad=NLpad, K=K, HALF=HALF,
                W_lo=W_lo, W_hi=W_hi, S=S, segs=segs, perms=perms,
                pad_frac=float(pad_frac))
    return in_maps, meta


def host_consts(W0, b0, W1, b1, gamma0, beta0, mean0, var0,
                gamma1, beta1, mean1, var1, act_params):
    eps = 1e-5
    s0 = (gamma0 / np.sqrt(var0 + eps)).astype(np.float32)
    s1 = (gamma1 / np.sqrt(var1 + eps)).astype(np.float32)
    c0 = ((b0 - mean0) * s0 + beta0).astype(np.float32)
    c1 = ((b1 - mean1) * s1 + beta1).astype(np.float32)
    crow = np.concatenate([c0, c1]).reshape(1, 256)
    alpha = float(1.0 / (1.0 + np.exp(-float(np.asarray(act_params).reshape(-1)[0]))))
    ident = np.eye(128, dtype=np.float32)
    iota = np.tile(np.arange(128, dtype=np.float32)[None, :], (128, 1)).astype(ml_dtypes.bfloat16)
    return {
        "w0": (W0 * s0[None, :]).astype(np.float32),
        "w1": (W1 * s1[None, :]).astype(np.float32),
        "crow": crow,
        "ident": ident,
        "iota": iota,
    }, alpha


# ---------------------------------------------------------------- builder

def build(meta, cfg: Cfg):
    NL, NB, NLpad = meta["NL"], meta["NB"], meta["NLpad"]
    K, HALF = meta["K"], meta["HALF"]
    W_lo, W_hi, S = meta["W_lo"], meta["W_hi"], meta["S"]
    segs = meta["segs"]
    N, P, GS = cfg.N, cfg.P, cfg.GS
    alpha = float(meta["alpha"])
    gelu_fn = AF.Gelu if cfg.gelu_hw else AF.Sigmoid

    nc = bacc.Bacc(None, target_bir_lowering=False, debug=False)

    xT_ext = nc.declare_dram_parameter("xT", [128, NLpad], F32, isOutput=False)
    idxlo_ext = nc.declare_dram_parameter("idxlo", [128, 8 * max(W_lo, 1)], I16, isOutput=False)
    idxhi_ext = nc.declare_dram_parameter("idxhi", [128, 8 * max(W_hi, 1)], I16, isOutput=False)
    dstoff_ext = nc.declare_dram_parameter("dstoff", [128, S], F32, isOutput=False)
    ewc_ext = nc.declare_dram_parameter("ewc", [128, S], F32, isOutput=False)
    ewrow_ext = nc.declare_dram_parameter("ewrow", [128, NB * K], F32, isOutput=False)
    w0_ext = nc.declare_dram_parameter("w0", [128, 128], F32, isOutput=False)
    w1_ext = nc.declare_dram_parameter("w1", [128, 128], F32, isOutput=False)
    crow_ext = nc.declare_dram_parameter("crow", [1, 256], F32, isOutput=False)
    ident_ext = nc.declare_dram_parameter("ident", [128, 128], F32, isOutput=False)
    iota_ext = nc.declare_dram_parameter("iota", [128, 128], BF16, isOutput=False)
    out_ext = nc.declare_dram_parameter("out", [NL, 128], F32, isOutput=True)

    hs_loc = nc.dram_tensor("hs_loc", [NL, 128], BF16)
    hs_full = nc.dram_tensor("hs_full", [N, 128], BF16, addr_space="Shared")
    hs2_loc = nc.dram_tensor("hs2_loc", [NL, 128], BF16)
    hs2_full = nc.dram_tensor("hs2_full", [N, 128], BF16, addr_space="Shared")

    groups = [list(range(P))]

    with tile.TileContext(nc, num_cores=P) as tc, ExitStack() as ctx:
        nc.gpsimd.load_library(library_config.mlp)
        cst = ctx.enter_context(tc.tile_pool(name="cst", bufs=1))
        w0_sb = cst.tile([128, 128], F32)
        w1_sb = cst.tile([128, 128], F32)
        crow_sb = cst.tile([1, 256], F32)
        ident_sb = cst.tile([128, 128], F32)
        xT_region = cst.tile([128, NLpad], F32)
        iota_sb = cst.tile([128, 128], BF16)
        ones_row = cst.tile([1, 128], F32)
        idxlo_sb = cst.tile([128, 8 * max(W_lo, 1)], I16)
        idxhi_sb = cst.tile([128, 8 * max(W_hi, 1)], I16)
        dstoff_sb = cst.tile([128, S], F32)
        ewc_sb = cst.tile([128, S], F32)
        ewrow_sb = cst.tile([128, NB * K], F32)
        deg_sb = cst.tile([128, NB], F32)
        dinv_sb = cst.tile([128, NB], F32)
        c0_rep = cst.tile([128, 128], F32)
        c1_rep = cst.tile([128, 128], F32)
        y1_region = cst.tile([128, NB * 128], F32)
        hs_region = cst.tile([128, NB * 128], BF16)
        hs2_region = cst.tile([128, NB * 128], BF16)

        nc.sync.dma_start(out=w0_sb[:, :], in_=w0_ext[:, :])
        nc.sync.dma_start(out=w1_sb[:, :], in_=w1_ext[:, :])
        nc.sync.dma_start(out=crow_sb[:, :], in_=crow_ext[:, :])
        nc.sync.dma_start(out=xT_region[:, :], in_=xT_ext[:, :])
        nc.sync.dma_start(out=ident_sb[:, :], in_=ident_ext[:, :])
        nc.sync.dma_start(out=iota_sb[:, :], in_=iota_ext[:, :])
        nc.sync.dma_start(out=idxlo_sb[:, :], in_=idxlo_ext[:, :])
        nc.sync.dma_start(out=idxhi_sb[:, :], in_=idxhi_ext[:, :])
        nc.sync.dma_start(out=dstoff_sb[:, :], in_=dstoff_ext[:, :])
        nc.sync.dma_start(out=ewc_sb[:, :], in_=ewc_ext[:, :])
        nc.sync.dma_start(out=ewrow_sb[:, :], in_=ewrow_ext[:, :])
        nc.vector.memset(ones_row[:, :], 1.0)

        mpool = ctx.enter_context(tc.tile_pool(name="mh", bufs=12))
        gpool = ctx.enter_context(tc.tile_pool(name="gh", bufs=6))
        psm = ctx.enter_context(tc.tile_pool(name="ps", bufs=6, space="PSUM"))
        wk = ctx.enter_context(tc.tile_pool(name="wk", bufs=4))
        psh = ctx.enter_context(tc.tile_pool(name="psh", bufs=1, space="PSUM"))
        pst = ctx.enter_context(tc.tile_pool(name="pst", bufs=1, space="PSUM"))
        sbp = ctx.enter_context(tc.tile_pool(name="sbp", bufs=3))

        def build_m(si, pool):
            m = pool.tile([128, 128], BF16, tag="m")
            nc.vector.tensor_scalar(m[:, :], iota_sb[:, :],
                                    dstoff_sb[:, si:si + 1], ewc_sb[:, si:si + 1],
                                    OP.is_equal, OP.mult)
            return m

        # deg via one reduction of the per-dst-row ew layout; +1 self-loop
        nc.vector.tensor_reduce(
            deg_sb[:, :], ewrow_sb[:, :].rearrange("p (b k) -> p b k", k=K),
            mybir.AxisListType.X, OP.add)
        nc.vector.tensor_scalar_add(deg_sb[:, :], deg_sb[:, :], 1.0)
        nc.scalar.activation(dinv_sb[:, :], deg_sb[:, :], AF.Sqrt)
        nc.vector.tensor_scalar_max(dinv_sb[:, :], dinv_sb[:, :], 0.5)
        nc.vector.reciprocal(dinv_sb[:, :], dinv_sb[:, :])

        for i, rep in ((0, c0_rep), (1, c1_rep)):
            pr = psh.tile([128, 128], F32, tag="h")
            nc.tensor.matmul(pr[:, :], ones_row[:, :], crow_sb[0:1, i * 128:(i + 1) * 128])
            nc.scalar.activation(rep[:, :], pr[:, :], AF.Copy)

        def table_block_l1(t):
            col = slice(t * 128, (t + 1) * 128)
            ph = psh.tile([128, 128], F32, tag="h")
            nc.tensor.matmul(ph[:, :], xT_region[:, col], w0_sb[:, :])
            nc.vector.tensor_scalar(hs_region[:, col], ph[:, :],
                                    dinv_sb[:, t:t + 1], None, OP.mult)

        def table_block_l2(t):
            col = slice(t * 128, (t + 1) * 128)
            ptr_ = pst.tile([128, 128], F32, tag="tr")
            nc.tensor.transpose(ptr_[:, :], y1_region[:, col], ident_sb[:, :])
            lhsT = sbp.tile([128, 128], F32, tag="lhs")
            nc.vector.tensor_copy(lhsT[:, :], ptr_[:, :])
            ph = psh.tile([128, 128], F32, tag="h")
            nc.tensor.matmul(ph[:, :], lhsT[:, :], w1_sb[:, :])
            nc.vector.tensor_scalar(hs2_region[:, col], ph[:, :],
                                    dinv_sb[:, t:t + 1], None, OP.mult)

        def _store_region(region, loc):
            full_nb = NL // 128
            rem = NL - full_nb * 128
            if full_nb:
                nc.sync.dma_start(
                    out=loc[0:full_nb * 128, :].rearrange("(b p) f -> p b f", p=128),
                    in_=region[:, 0:full_nb * 128].rearrange("p (b f) -> p b f", f=128))
            if rem:
                nc.sync.dma_start(
                    out=loc[full_nb * 128:NL, :],
                    in_=region[0:rem, full_nb * 128:(full_nb + 1) * 128])

        # segments grouped by block, in (half, window) order per block
        segs_of_block = [[] for _ in range(NB)]
        for si, (b, h, w, r0, r1) in enumerate(segs):
            segs_of_block[b].append((si, h, w))
        W_half = [W_lo, W_hi]

        def scatter_pass(table_full, post_fn):
            g_tiles = [{}, {}]
            idx_sb = [idxlo_sb, idxhi_sb]
            half_view = [table_full[0:HALF, :], table_full[HALF:N, :]]

            def g_window(h, w):
                ch = w // GS
                if ch not in g_tiles[h]:
                    lo = ch * GS
                    hi = min(W_half[h], lo + GS)
                    Sn = hi - lo
                    t_ = gpool.tile([128, Sn, 128], BF16, tag=f"gt{h}")
                    nc.gpsimd.dma_gather(
                        t_[:, :, :], half_view[h], idx_sb[h][:, lo * 8:hi * 8],
                        num_idxs=Sn * 128, num_idxs_reg=Sn * 128, elem_size=128)
                    g_tiles[h].clear()
                    g_tiles[h][ch] = (t_, lo)
                t_, lo = g_tiles[h][ch]
                return t_[:, w - lo, :]

            for b in range(NB):
                sl = segs_of_block[b]
                pm = psm.tile([128, 128], F32, tag="pm")
                for j, (si, h, w) in enumerate(sl):
                    m = build_m(si, mpool)
                    g_ap = g_window(h, w)
                    nc.tensor.matmul(pm[:, :], m[:, :], g_ap,
                                     start=(j == 0), stop=(j == len(sl) - 1))
                post_fn(b, pm, wk)

        # ---------------- layer 1
        for t in range(NB):
            table_block_l1(t)
        _store_region(hs_region, hs_loc)
        nc.gpsimd.collective_compute(
            "AllGather", OP.bypass, replica_groups=groups,
            ins=[hs_loc[:, :]], outs=[hs_full[:, :]])

        def post1(b, pm, wkp):
            col = slice(b * 128, (b + 1) * 128)
            u = wkp.tile([128, 128], F32, tag="u")
            nc.vector.tensor_add(u[:, :], pm[:, :], hs_region[:, col])
            nc.vector.tensor_scalar(u[:, :], u[:, :], dinv_sb[:, b:b + 1], None, OP.mult)
            nc.vector.tensor_add(u[:, :], u[:, :], c0_rep[:, :])
            r = wkp.tile([128, 128], F32, tag="r")
            g = wkp.tile([128, 128], F32, tag="g")
            nc.scalar.activation(r[:, :], u[:, :], AF.Relu)
            nc.scalar.activation(g[:, :], u[:, :], gelu_fn)
            nc.vector.tensor_scalar(r[:, :], r[:, :], alpha, None, OP.mult)
            nc.vector.tensor_scalar(g[:, :], g[:, :], 1.0 - alpha, None, OP.mult)
            nc.vector.tensor_add(y1_region[:, col], r[:, :], g[:, :])
            table_block_l2(b)

        scatter_pass(hs_full, post1)

        # ---------------- layer 2
        _store_region(hs2_region, hs2_loc)
        nc.gpsimd.collective_compute(
            "AllGather", OP.bypass, replica_groups=groups,
            ins=[hs2_loc[:, :]], outs=[hs2_full[:, :]])

        out_region = y1_region

        def post2(b, pm, wkp):
            col = slice(b * 128, (b + 1) * 128)
            u = wkp.tile([128, 128], F32, tag="u")
            nc.vector.tensor_add(u[:, :], pm[:, :], hs2_region[:, col])
            nc.vector.tensor_scalar(u[:, :], u[:, :], dinv_sb[:, b:b + 1], None, OP.mult)
            nc.vector.tensor_add(out_region[:, col], u[:, :], c1_rep[:, :])
            # stream this block's rows out as soon as they're final
            lo = b * 128
            hi = min(NL, lo + 128)
            nc.sync.dma_start(out=out_ext[lo:hi, :],
                              in_=out_region[0:hi - lo, col])

        scatter_pass(hs2_full, post2)

    nc.finalize()
    return nc


# ---------------------------------------------------------------- runners

def prep_all(inputs, cfg: Cfg):
    in_maps, meta = host_prep(inputs["x"], inputs["edge_index"],
                              inputs["edge_weight"], cfg)
    consts, alpha = host_consts(
        inputs["W0"], inputs["b0"], inputs["W1"], inputs["b1"],
        inputs["gamma0"], inputs["beta0"], inputs["mean0"],
        inputs["var0"], inputs["gamma1"], inputs["beta1"],
        inputs["mean1"], inputs["var1"], inputs["act_params"])
    meta["alpha"] = alpha
    for m in in_maps:
        m.update(consts)
    return in_maps, meta


def unshard(results, cfg: Cfg, meta=None):
    NL = cfg.N // cfg.P
    out = np.zeros((cfg.N, cfg.D), np.float32)
    for c in range(cfg.P):
        r = results[c]["out"]
        if meta is not None and "perms" in meta:
            out[c * NL:(c + 1) * NL] = r[meta["perms"][c]]
        else:
            out[c * NL:(c + 1) * NL] = r
    return out


# ---------------------------------------------------------------- entrypoint

def _install_dge_patch():
    """walrus needs --dge-levels=vector_dynamic_offsets for the indirect
    (DynamicAP) gather DMAs this kernel uses."""
    from concourse import bass_utils as _bu
    if getattr(_bu, "_gcn_dge_patched", False):
        return
    _orig = _bu.run_command

    def _patched(argv, **kwargs):
        if argv and "walrus_driver" in str(argv[0]) and not any(
                str(a).startswith("--dge-levels") for a in argv):
            argv = list(argv) + ["--dge-levels=vector_dynamic_offsets"]
        return _orig(argv, **kwargs)

    _bu.run_command = _patched
    _bu._gcn_dge_patched = True


_CFG = Cfg()


def kernel(**inputs):
    """Full-input entrypoint: shard, run on 8 NeuronCores, gather output."""
    import numpy as _np
    _install_dge_patch()
    inputs = {k: _np.asarray(v) for k, v in inputs.items()}
    in_maps, meta = prep_all(inputs, _CFG)
    nc = build(meta, _CFG)
    res = run_bass_kernel_spmd(nc, in_maps, core_ids=list(range(_CFG.P)))
    return unshard([{k: _np.asarray(v) for k, v in r.items()} for r in res.results],
                   _CFG, meta)


# revision 4
# speedup vs baseline: 1.0734x; 1.0037x over previous
"""AdaptiveGCN v12 (split AG, lo/hi sub-phases): packed gather windows (no per-block 128-ceil).

Edges per core sorted by (half, block, dst). Per (block, half) padded to the
max count over cores (uniform schedule), then the padded stream is chopped
into 128-row gather windows IGNORING block boundaries. A window crossing a
block boundary is consumed by one matmul per touched block ("segment"), with
the one-hot M zero outside the segment's row range (host zeroes ew there).
Gather calls = windows/8 per half -> ~12% fewer Pool-serialized calls than
per-block slot padding, and gather DMA bytes drop by the pad delta.

Also: deg via per-dst-row ew reduce; self-loops folded as local table add;
BN scale folded into W on host; layer-2 table built inside post1.
"""

import dataclasses
import ml_dtypes
import numpy as np
from contextlib import ExitStack

from concourse import bass, bacc, mybir, tile, library_config
from concourse.bass_utils import run_bass_kernel_spmd

F32 = mybir.dt.float32
BF16 = mybir.dt.bfloat16
I32 = mybir.dt.int32
I16 = mybir.dt.int16
AF = mybir.ActivationFunctionType
OP = mybir.AluOpType


@dataclasses.dataclass
class Cfg:
    N: int = 50000
    E: int = 600000
    D: int = 128
    P: int = 8
    BLK: int = 128
    GS: int = 8           # windows per dma_gather call (1024-idx ucode cap)
    bn_eps: float = 1e-5
    gelu_hw: bool = True


# ---------------------------------------------------------------- host prep

def host_prep(x, edge_index, edge_weight, cfg: Cfg):
    N, E, P, BLK = cfg.N, cfg.E, cfg.P, cfg.BLK
    assert x.shape == (N, cfg.D) and cfg.D == 128
    NL = N // P
    NB = (NL + BLK - 1) // BLK
    NLpad = NB * BLK

    src = edge_index[0].astype(np.int64)
    dst = edge_index[1].astype(np.int64)
    ew = edge_weight.astype(np.float32)

    core_of = dst // NL
    import heapq
    indeg = np.zeros(N, np.int64)
    np.add.at(indeg, dst, 1)
    perms = []
    for c in range(P):
        deg_c = indeg[c * NL:(c + 1) * NL]
        order_n = np.argsort(-deg_c, kind="stable")
        cap = [BLK] * NB
        cap[NB - 1] = NL - BLK * (NB - 1)
        heap = [(0, bi) for bi in range(NB)]
        heapq.heapify(heap)
        fill = [0] * NB
        pnew = np.zeros(NL, np.int64)
        for ln in order_n:
            while True:
                tot, bi = heapq.heappop(heap)
                if fill[bi] < cap[bi]:
                    break
            pnew[ln] = bi * BLK + fill[bi]
            fill[bi] += 1
            if fill[bi] < cap[bi]:
                heapq.heappush(heap, (tot + int(deg_c[ln]), bi))
        perms.append(pnew)

    # table row layout: per-core low rows (l < LA) rank-interleaved first,
    # then per-core high rows; both halves are contiguous -> each can be
    # AllGathered independently and gathered from with int16 offsets.
    LA = 3072          # 24 blocks; must be a multiple of BLK
    LB = NL - LA
    HALF = P * LA      # 24576
    assert HALF <= 32767 and (N - HALF) <= 32767
    perm_all = np.concatenate(perms)
    core_all = np.arange(N) // NL
    tpos_of = np.where(perm_all < LA,
                       core_all * LA + perm_all,
                       HALF + core_all * LB + (perm_all - LA))

    per_core = []
    counts = np.zeros((P, NB, 2), np.int64)
    K = 1
    for c in range(P):
        m = core_of == c
        s, dl, w = tpos_of[src[m]], perms[c][dst[m] - c * NL], ew[m]
        hh = (s >= HALF).astype(np.int64)
        b = dl // BLK
        order = np.lexsort((dl, b, hh))   # half outermost
        s, dl, w, hh, b = s[order], dl[order], w[order], hh[order], b[order]
        per_core.append((s, dl, w, hh, b))
        for bi in range(NB):
            mb_ = b == bi
            counts[c, bi, 0] = np.sum(mb_ & (hh == 0))
            counts[c, bi, 1] = np.sum(mb_ & (hh == 1))
        cnt_n = np.bincount(dl, minlength=NLpad)
        K = max(K, int(cnt_n.max()))

    cmax = counts.max(axis=0)          # [NB, 2] uniform padded counts
    # windows per half; stream order within half h: blocks 0..NB-1
    W_h = []
    pref = np.zeros((2, NB + 1), np.int64)
    for h in (0, 1):
        pref[h, 1:] = np.cumsum(cmax[:, h])
        W_h.append(int(-(-pref[h, NB] // 128)))
    W_lo, W_hi = W_h
    Wtot = W_lo + W_hi

    # segments (matmul units): per half, per block, windows it spans
    segs = []   # (b, h, w, r0, r1)  rows [r0, r1) within window w (half-local)
    for h in (0, 1):
        for b in range(NB):
            p0, p1 = int(pref[h, b]), int(pref[h, b + 1])
            if p1 == p0:
                continue
            wlo, whi = p0 // 128, (p1 - 1) // 128
            for w in range(wlo, whi + 1):
                r0 = max(0, p0 - w * 128)
                r1 = min(128, p1 - w * 128)
                segs.append((b, h, w, r0, r1))
    S = len(segs)
    real_edges = counts.sum()
    pad_frac = (Wtot * 128 * P - real_edges) / max(real_edges, 1)

    in_maps = []
    for c in range(P):
        s, dl, w, hh, b = per_core[c]
        ewrow = np.zeros((128, NB * K), np.float32)
        fill_n = np.zeros(NLpad, np.int64)
        for e in range(len(dl)):
            n = dl[e]
            ewrow[n % 128, (n // 128) * K + fill_n[n]] = w[e]
            fill_n[n] += 1

        idx = [np.zeros((16, 8 * max(W_lo, 1)), np.int16),
               np.zeros((16, 8 * max(W_hi, 1)), np.int16)]
        dstoff = np.zeros((128, S), np.float32)
        ewc = np.zeros((128, S), np.float32)

        # place this core's edges at padded-stream positions, then re-sort
        # each 128-row window by src table row (HBM locality for the gather;
        # the one-hot M absorbs any within-window permutation)
        ptr = 0
        edge_pos = {}   # (h, stream_pos) -> edge index; only real edges
        for h in (0, 1):
            for bi in range(NB):
                cnt = int(counts[c, bi, h])
                base = int(pref[h, bi])
                for k in range(cnt):
                    edge_pos[(h, base + k)] = ptr + k
                ptr += cnt
        for h in (0, 1):
            for wdx in range(W_h[h]):
                rows = [(h, wdx * 128 + j) for j in range(128)]
                es = [edge_pos.get(r) for r in rows]
                keyed = sorted((e for e in es if e is not None), key=lambda e: s[e])
                npad = sum(1 for e in es if e is None)
                for j, r in enumerate(rows):
                    if j < len(keyed):
                        edge_pos[r] = keyed[j]
                    elif r in edge_pos:
                        del edge_pos[r]
        # idx arrays per window
        for h in (0, 1):
            for wdx in range(W_h[h]):
                for j in range(128):
                    e = edge_pos.get((h, wdx * 128 + j))
                    ii = 0 if e is None else int(s[e] - h * HALF)
                    idx[h][j % 16, wdx * 8 + j // 16] = ii
        # per-seg columns by block membership (rows are permuted within
        # windows, so an edge can sit outside its block's contiguous range)
        seg_of = {(bi, h, wdx): si for si, (bi, h, wdx, r0, r1) in enumerate(segs)}
        for h in (0, 1):
            for wdx in range(W_h[h]):
                for j in range(128):
                    e = edge_pos.get((h, wdx * 128 + j))
                    if e is not None:
                        bb = int(dl[e]) // BLK
                        si = seg_of[(bb, h, wdx)]
                        dstoff[j, si] = float(dl[e] - bb * BLK)
                        ewc[j, si] = w[e]

        xT = np.zeros((128, NLpad), np.float32)
        xT[:, perms[c]] = x[c * NL:(c + 1) * NL].T
        in_maps.append({
            "xT": xT,
            "idxlo": np.tile(idx[0], (8, 1)),
            "idxhi": np.tile(idx[1], (8, 1)),
            "dstoff": dstoff,
            "ewc": ewc,
            "ewrow": ewrow,
        })

    meta = dict(NL=NL, NB=NB, NLpad=NLpad, K=K, HALF=HALF, LA=LA, LB=LB,
                W_lo=W_lo, W_hi=W_hi, S=S, segs=segs, perms=perms,
                pad_frac=float(pad_frac))
    return in_maps, meta


def host_consts(W0, b0, W1, b1, gamma0, beta0, mean0, var0,
                gamma1, beta1, mean1, var1, act_params):
    eps = 1e-5
    s0 = (gamma0 / np.sqrt(var0 + eps)).astype(np.float32)
    s1 = (gamma1 / np.sqrt(var1 + eps)).astype(np.float32)
    c0 = ((b0 - mean0) * s0 + beta0).astype(np.float32)
    c1 = ((b1 - mean1) * s1 + beta1).astype(np.float32)
    crow = np.concatenate([c0, c1]).reshape(1, 256)
    alpha = float(1.0 / (1.0 + np.exp(-float(np.asarray(act_params).reshape(-1)[0]))))
    ident = np.eye(128, dtype=np.float32)
    iota = np.tile(np.arange(128, dtype=np.float32)[None, :], (128, 1)).astype(ml_dtypes.bfloat16)
    return {
        "w0": (W0 * s0[None, :]).astype(np.float32),
        "w1": (W1 * s1[None, :]).astype(np.float32),
        "crow": crow,
        "ident": ident,
        "iota": iota,
    }, alpha


# ---------------------------------------------------------------- builder

def build(meta, cfg: Cfg):
    NL, NB, NLpad = meta["NL"], meta["NB"], meta["NLpad"]
    K, HALF = meta["K"], meta["HALF"]
    LA, LB = meta["LA"], meta["LB"]
    NBA = LA // 128              # blocks in the low piece
    W_lo, W_hi, S = meta["W_lo"], meta["W_hi"], meta["S"]
    segs = meta["segs"]
    N, P, GS = cfg.N, cfg.P, cfg.GS
    alpha = float(meta["alpha"])
    gelu_fn = AF.Gelu if cfg.gelu_hw else AF.Sigmoid

    nc = bacc.Bacc(None, target_bir_lowering=False, debug=False)

    xT_ext = nc.declare_dram_parameter("xT", [128, NLpad], F32, isOutput=False)
    idxlo_ext = nc.declare_dram_parameter("idxlo", [128, 8 * max(W_lo, 1)], I16, isOutput=False)
    idxhi_ext = nc.declare_dram_parameter("idxhi", [128, 8 * max(W_hi, 1)], I16, isOutput=False)
    dstoff_ext = nc.declare_dram_parameter("dstoff", [128, S], F32, isOutput=False)
    ewc_ext = nc.declare_dram_parameter("ewc", [128, S], F32, isOutput=False)
    ewrow_ext = nc.declare_dram_parameter("ewrow", [128, NB * K], F32, isOutput=False)
    w0_ext = nc.declare_dram_parameter("w0", [128, 128], F32, isOutput=False)
    w1_ext = nc.declare_dram_parameter("w1", [128, 128], F32, isOutput=False)
    crow_ext = nc.declare_dram_parameter("crow", [1, 256], F32, isOutput=False)
    ident_ext = nc.declare_dram_parameter("ident", [128, 128], F32, isOutput=False)
    iota_ext = nc.declare_dram_parameter("iota", [128, 128], BF16, isOutput=False)
    out_ext = nc.declare_dram_parameter("out", [NL, 128], F32, isOutput=True)

    hs_loca = nc.dram_tensor("hs_loca", [LA, 128], BF16)
    hs_locb = nc.dram_tensor("hs_locb", [LB, 128], BF16)
    hs_full = nc.dram_tensor("hs_full", [N, 128], BF16, addr_space="Shared")
    hs2_loca = nc.dram_tensor("hs2_loca", [LA, 128], BF16)
    hs2_locb = nc.dram_tensor("hs2_locb", [LB, 128], BF16)
    hs2_full = nc.dram_tensor("hs2_full", [N, 128], BF16, addr_space="Shared")

    groups = [list(range(P))]

    with tile.TileContext(nc, num_cores=P) as tc, ExitStack() as ctx:
        nc.gpsimd.load_library(library_config.mlp)
        cst = ctx.enter_context(tc.tile_pool(name="cst", bufs=1))
        w0_sb = cst.tile([128, 128], F32)
        w1_sb = cst.tile([128, 128], F32)
        crow_sb = cst.tile([1, 256], F32)
        ident_sb = cst.tile([128, 128], F32)
        xT_region = cst.tile([128, NLpad], F32)
        iota_sb = cst.tile([128, 128], BF16)
        ones_row = cst.tile([1, 128], F32)
        idxlo_sb = cst.tile([128, 8 * max(W_lo, 1)], I16)
        idxhi_sb = cst.tile([128, 8 * max(W_hi, 1)], I16)
        dstoff_sb = cst.tile([128, S], F32)
        ewc_sb = cst.tile([128, S], F32)
        ewrow_sb = cst.tile([128, NB * K], F32)
        deg_sb = cst.tile([128, NB], F32)
        dinv_sb = cst.tile([128, NB], F32)
        c0_rep = cst.tile([128, 128], F32)
        c1_rep = cst.tile([128, 128], F32)
        y1_region = cst.tile([128, NB * 128], F32)
        part_region = cst.tile([128, NB * 128], F32)
        hs_region = cst.tile([128, NB * 128], BF16)
        hs2_region = cst.tile([128, NB * 128], BF16)

        nc.sync.dma_start(out=w0_sb[:, :], in_=w0_ext[:, :])
        nc.sync.dma_start(out=w1_sb[:, :], in_=w1_ext[:, :])
        nc.sync.dma_start(out=crow_sb[:, :], in_=crow_ext[:, :])
        nc.sync.dma_start(out=xT_region[:, :], in_=xT_ext[:, :])
        nc.sync.dma_start(out=ident_sb[:, :], in_=ident_ext[:, :])
        nc.sync.dma_start(out=iota_sb[:, :], in_=iota_ext[:, :])
        nc.sync.dma_start(out=idxlo_sb[:, :], in_=idxlo_ext[:, :])
        nc.sync.dma_start(out=idxhi_sb[:, :], in_=idxhi_ext[:, :])
        nc.sync.dma_start(out=dstoff_sb[:, :], in_=dstoff_ext[:, :])
        nc.sync.dma_start(out=ewc_sb[:, :], in_=ewc_ext[:, :])
        nc.sync.dma_start(out=ewrow_sb[:, :], in_=ewrow_ext[:, :])
        nc.vector.memset(ones_row[:, :], 1.0)

        mpool = ctx.enter_context(tc.tile_pool(name="mh", bufs=12))
        gpool = ctx.enter_context(tc.tile_pool(name="gh", bufs=6))
        psm = ctx.enter_context(tc.tile_pool(name="ps", bufs=6, space="PSUM"))
        wk = ctx.enter_context(tc.tile_pool(name="wk", bufs=4))
        psh = ctx.enter_context(tc.tile_pool(name="psh", bufs=1, space="PSUM"))
        pst = ctx.enter_context(tc.tile_pool(name="pst", bufs=1, space="PSUM"))
        sbp = ctx.enter_context(tc.tile_pool(name="sbp", bufs=3))

        def build_m(si, pool):
            m = pool.tile([128, 128], BF16, tag="m")
            nc.vector.tensor_scalar(m[:, :], iota_sb[:, :],
                                    dstoff_sb[:, si:si + 1], ewc_sb[:, si:si + 1],
                                    OP.is_equal, OP.mult)
            return m

        # deg via one reduction of the per-dst-row ew layout; +1 self-loop
        nc.vector.tensor_reduce(
            deg_sb[:, :], ewrow_sb[:, :].rearrange("p (b k) -> p b k", k=K),
            mybir.AxisListType.X, OP.add)
        nc.vector.tensor_scalar_add(deg_sb[:, :], deg_sb[:, :], 1.0)
        nc.scalar.activation(dinv_sb[:, :], deg_sb[:, :], AF.Sqrt)
        nc.vector.tensor_scalar_max(dinv_sb[:, :], dinv_sb[:, :], 0.5)
        nc.vector.reciprocal(dinv_sb[:, :], dinv_sb[:, :])

        for i, rep in ((0, c0_rep), (1, c1_rep)):
            pr = psh.tile([128, 128], F32, tag="h")
            nc.tensor.matmul(pr[:, :], ones_row[:, :], crow_sb[0:1, i * 128:(i + 1) * 128])
            nc.scalar.activation(rep[:, :], pr[:, :], AF.Copy)

        def table_block_l1(t):
            col = slice(t * 128, (t + 1) * 128)
            ph = psh.tile([128, 128], F32, tag="h")
            nc.tensor.matmul(ph[:, :], xT_region[:, col], w0_sb[:, :])
            nc.vector.tensor_scalar(hs_region[:, col], ph[:, :],
                                    dinv_sb[:, t:t + 1], None, OP.mult)

        def table_block_l2(t):
            col = slice(t * 128, (t + 1) * 128)
            ptr_ = pst.tile([128, 128], F32, tag="tr")
            nc.tensor.transpose(ptr_[:, :], y1_region[:, col], ident_sb[:, :])
            lhsT = sbp.tile([128, 128], F32, tag="lhs")
            nc.vector.tensor_copy(lhsT[:, :], ptr_[:, :])
            ph = psh.tile([128, 128], F32, tag="h")
            nc.tensor.matmul(ph[:, :], lhsT[:, :], w1_sb[:, :])
            nc.vector.tensor_scalar(hs2_region[:, col], ph[:, :],
                                    dinv_sb[:, t:t + 1], None, OP.mult)

        def _store_piece(region, loc, b0, nrows):
            # store region cols [b0*128, b0*128+nrows) -> loc[0:nrows]
            full_nb = nrows // 128
            rem = nrows - full_nb * 128
            c0 = b0 * 128
            if full_nb:
                nc.sync.dma_start(
                    out=loc[0:full_nb * 128, :].rearrange("(b p) f -> p b f", p=128),
                    in_=region[:, c0:c0 + full_nb * 128].rearrange("p (b f) -> p b f", f=128))
            if rem:
                nc.sync.dma_start(
                    out=loc[full_nb * 128:nrows, :],
                    in_=region[0:rem, c0 + full_nb * 128:c0 + (full_nb + 1) * 128])

        # segments grouped by block, in (half, window) order per block
        segs_of_block = [[] for _ in range(NB)]
        for si, (b, h, w, r0, r1) in enumerate(segs):
            segs_of_block[b].append((si, h, w))
        W_half = [W_lo, W_hi]

        def scatter_pass(table_full, post_fn):
            # lo sub-phase needs only table rows [0, HALF) (AG piece a);
            # hi sub-phase needs [HALF, N) (piece b). Per block: lo segs
            # accumulate into part_region, hi segs re-accumulate in PSUM and
            # the post adds both.
            g_tiles = [{}, {}]
            idx_sb = [idxlo_sb, idxhi_sb]
            half_view = [table_full[0:HALF, :], table_full[HALF:N, :]]

            def g_window(h, w):
                ch = w // GS
                if ch not in g_tiles[h]:
                    lo = ch * GS
                    hi = min(W_half[h], lo + GS)
                    Sn = hi - lo
                    t_ = gpool.tile([128, Sn, 128], BF16, tag=f"gt{h}")
                    nc.gpsimd.dma_gather(
                        t_[:, :, :], half_view[h], idx_sb[h][:, lo * 8:hi * 8],
                        num_idxs=Sn * 128, num_idxs_reg=Sn * 128, elem_size=128)
                    g_tiles[h].clear()
                    g_tiles[h][ch] = (t_, lo)
                t_, lo = g_tiles[h][ch]
                return t_[:, w - lo, :]

            for b in range(NB):
                sl = [sg for sg in segs_of_block[b] if sg[1] == 0]
                col = slice(b * 128, (b + 1) * 128)
                if sl:
                    pm = psm.tile([128, 128], F32, tag="pm")
                    for j, (si, h, w) in enumerate(sl):
                        m = build_m(si, mpool)
                        g_ap = g_window(h, w)
                        nc.tensor.matmul(pm[:, :], m[:, :], g_ap,
                                         start=(j == 0), stop=(j == len(sl) - 1))
                    nc.scalar.activation(part_region[:, col], pm[:, :], AF.Copy)
                else:
                    nc.vector.memset(part_region[:, col], 0.0)
            for b in range(NB):
                sl = [sg for sg in segs_of_block[b] if sg[1] == 1]
                col = slice(b * 128, (b + 1) * 128)
                pm = psm.tile([128, 128], F32, tag="pm")
                if sl:
                    for j, (si, h, w) in enumerate(sl):
                        m = build_m(si, mpool)
                        g_ap = g_window(h, w)
                        nc.tensor.matmul(pm[:, :], m[:, :], g_ap,
                                         start=(j == 0), stop=(j == len(sl) - 1))
                    nc.vector.tensor_add(part_region[:, col], part_region[:, col],
                                         pm[:, :])
                post_fn(b, wk)

        # ---------------- layer 1 (piece a first so its AG fires early)
        for t in range(NBA):
            table_block_l1(t)
        _store_piece(hs_region, hs_loca, 0, LA)
        nc.gpsimd.collective_compute(
            "AllGather", OP.bypass, replica_groups=groups,
            ins=[hs_loca[:, :]], outs=[hs_full[0:HALF, :]])
        for t in range(NBA, NB):
            table_block_l1(t)
        _store_piece(hs_region, hs_locb, NBA, LB)
        nc.gpsimd.collective_compute(
            "AllGather", OP.bypass, replica_groups=groups,
            ins=[hs_locb[:, :]], outs=[hs_full[HALF:N, :]])

        def post1(b, wkp):
            col = slice(b * 128, (b + 1) * 128)
            u = wkp.tile([128, 128], F32, tag="u")
            nc.vector.tensor_add(u[:, :], part_region[:, col], hs_region[:, col])
            nc.vector.tensor_scalar(u[:, :], u[:, :], dinv_sb[:, b:b + 1], None, OP.mult)
            nc.vector.tensor_add(u[:, :], u[:, :], c0_rep[:, :])
            r = wkp.tile([128, 128], F32, tag="r")
            g = wkp.tile([128, 128], F32, tag="g")
            nc.scalar.activation(r[:, :], u[:, :], AF.Relu)
            nc.scalar.activation(g[:, :], u[:, :], gelu_fn)
            nc.vector.tensor_scalar(r[:, :], r[:, :], alpha, None, OP.mult)
            nc.vector.tensor_scalar(g[:, :], g[:, :], 1.0 - alpha, None, OP.mult)
            nc.vector.tensor_add(y1_region[:, col], r[:, :], g[:, :])
            table_block_l2(b)
            if b == NBA - 1:
                _store_piece(hs2_region, hs2_loca, 0, LA)
                nc.gpsimd.collective_compute(
                    "AllGather", OP.bypass, replica_groups=groups,
                    ins=[hs2_loca[:, :]], outs=[hs2_full[0:HALF, :]])

        scatter_pass(hs_full, post1)

        # ---------------- layer 2
        _store_piece(hs2_region, hs2_locb, NBA, LB)
        nc.gpsimd.collective_compute(
            "AllGather", OP.bypass, replica_groups=groups,
            ins=[hs2_locb[:, :]], outs=[hs2_full[HALF:N, :]])

        out_region = y1_region

        def post2(b, wkp):
            col = slice(b * 128, (b + 1) * 128)
            u = wkp.tile([128, 128], F32, tag="u")
            nc.vector.tensor_add(u[:, :], part_region[:, col], hs2_region[:, col])
            nc.vector.tensor_scalar(u[:, :], u[:, :], dinv_sb[:, b:b + 1], None, OP.mult)
            nc.vector.tensor_add(out_region[:, col], u[:, :], c1_rep[:, :])
            # stream this block's rows out as soon as they're final
            lo = b * 128
            hi = min(NL, lo + 128)
            nc.sync.dma_start(out=out_ext[lo:hi, :],
                              in_=out_region[0:hi - lo, col])

        scatter_pass(hs2_full, post2)

    nc.finalize()
    return nc


# ---------------------------------------------------------------- runners

def prep_all(inputs, cfg: Cfg):
    in_maps, meta = host_prep(inputs["x"], inputs["edge_index"],
                              inputs["edge_weight"], cfg)
    consts, alpha = host_consts(
        inputs["W0"], inputs["b0"], inputs["W1"], inputs["b1"],
        inputs["gamma0"], inputs["beta0"], inputs["mean0"],
        inputs["var0"], inputs["gamma1"], inputs["beta1"],
        inputs["mean1"], inputs["var1"], inputs["act_params"])
    meta["alpha"] = alpha
    for m in in_maps:
        m.update(consts)
    return in_maps, meta


def unshard(results, cfg: Cfg, meta=None):
    NL = cfg.N // cfg.P
    out = np.zeros((cfg.N, cfg.D), np.float32)
    for c in range(cfg.P):
        r = results[c]["out"]
        if meta is not None and "perms" in meta:
            out[c * NL:(c + 1) * NL] = r[meta["perms"][c]]
        else:
            out[c * NL:(c + 1) * NL] = r
    return out


# ---------------------------------------------------------------- entrypoint

def _install_dge_patch():
    """walrus needs --dge-levels=vector_dynamic_offsets for the indirect
    (DynamicAP) gather DMAs this kernel uses."""
    from concourse import bass_utils as _bu
    if getattr(_bu, "_gcn_dge_patched", False):
        return
    _orig = _bu.run_command

    def _patched(argv, **kwargs):
        if argv and "walrus_driver" in str(argv[0]) and not any(
                str(a).startswith("--dge-levels") for a in argv):
            argv = list(argv) + ["--dge-levels=vector_dynamic_offsets"]
        return _orig(argv, **kwargs)

    _bu.run_command = _patched
    _bu._gcn_dge_patched = True


_CFG = Cfg()


def kernel(**inputs):
    """Full-input entrypoint: shard, run on 8 NeuronCores, gather output."""
    import numpy as _np
    _install_dge_patch()
    inputs = {k: _np.asarray(v) for k, v in inputs.items()}
    in_maps, meta = prep_all(inputs, _CFG)
    nc = build(meta, _CFG)
    res = run_bass_kernel_spmd(nc, in_maps, core_ids=list(range(_CFG.P)))
    return unshard([{k: _np.asarray(v) for k, v in r.items()} for r in res.results],
                   _CFG, meta)


# revision 5
# speedup vs baseline: 1.1469x; 1.0685x over previous
"""AdaptiveGCN v13 (bf16 tables, warmup cc): packed gather windows (no per-block 128-ceil).

Edges per core sorted by (half, block, dst). Per (block, half) padded to the
max count over cores (uniform schedule), then the padded stream is chopped
into 128-row gather windows IGNORING block boundaries. A window crossing a
block boundary is consumed by one matmul per touched block ("segment"), with
the one-hot M zero outside the segment's row range (host zeroes ew there).
Gather calls = windows/8 per half -> ~12% fewer Pool-serialized calls than
per-block slot padding, and gather DMA bytes drop by the pad delta.

Also: deg via per-dst-row ew reduce; self-loops folded as local table add;
BN scale folded into W on host; layer-2 table built inside post1.
"""

import dataclasses
import ml_dtypes
import numpy as np
from contextlib import ExitStack

from concourse import bass, bacc, mybir, tile, library_config
from concourse.bass_utils import run_bass_kernel_spmd

F32 = mybir.dt.float32
BF16 = mybir.dt.bfloat16
I32 = mybir.dt.int32
I16 = mybir.dt.int16
AF = mybir.ActivationFunctionType
OP = mybir.AluOpType


@dataclasses.dataclass
class Cfg:
    N: int = 50000
    E: int = 600000
    D: int = 128
    P: int = 8
    BLK: int = 128
    GS: int = 8           # windows per dma_gather call (1024-idx ucode cap)
    bn_eps: float = 1e-5
    gelu_hw: bool = True


# ---------------------------------------------------------------- host prep

def host_prep(x, edge_index, edge_weight, cfg: Cfg):
    N, E, P, BLK = cfg.N, cfg.E, cfg.P, cfg.BLK
    assert x.shape == (N, cfg.D) and cfg.D == 128
    NL = N // P
    NB = (NL + BLK - 1) // BLK
    NLpad = NB * BLK

    src = edge_index[0].astype(np.int64)
    dst = edge_index[1].astype(np.int64)
    ew = edge_weight.astype(np.float32)

    core_of = dst // NL
    import heapq
    indeg = np.zeros(N, np.int64)
    np.add.at(indeg, dst, 1)
    perms = []
    for c in range(P):
        deg_c = indeg[c * NL:(c + 1) * NL]
        order_n = np.argsort(-deg_c, kind="stable")
        cap = [BLK] * NB
        cap[NB - 1] = NL - BLK * (NB - 1)
        heap = [(0, bi) for bi in range(NB)]
        heapq.heapify(heap)
        fill = [0] * NB
        pnew = np.zeros(NL, np.int64)
        for ln in order_n:
            while True:
                tot, bi = heapq.heappop(heap)
                if fill[bi] < cap[bi]:
                    break
            pnew[ln] = bi * BLK + fill[bi]
            fill[bi] += 1
            if fill[bi] < cap[bi]:
                heapq.heappush(heap, (tot + int(deg_c[ln]), bi))
        perms.append(pnew)

    # table row layout: per-core low rows (l < LA) rank-interleaved first,
    # then per-core high rows; both halves are contiguous -> each can be
    # AllGathered independently and gathered from with int16 offsets.
    LA = 3072          # 24 blocks; must be a multiple of BLK
    LB = NL - LA
    HALF = P * LA      # 24576
    assert HALF <= 32767 and (N - HALF) <= 32767
    perm_all = np.concatenate(perms)
    core_all = np.arange(N) // NL
    tpos_of = np.where(perm_all < LA,
                       core_all * LA + perm_all,
                       HALF + core_all * LB + (perm_all - LA))

    per_core = []
    counts = np.zeros((P, NB, 2), np.int64)
    K = 1
    for c in range(P):
        m = core_of == c
        s, dl, w = tpos_of[src[m]], perms[c][dst[m] - c * NL], ew[m]
        hh = (s >= HALF).astype(np.int64)
        b = dl // BLK
        order = np.lexsort((dl, b, hh))   # half outermost
        s, dl, w, hh, b = s[order], dl[order], w[order], hh[order], b[order]
        per_core.append((s, dl, w, hh, b))
        for bi in range(NB):
            mb_ = b == bi
            counts[c, bi, 0] = np.sum(mb_ & (hh == 0))
            counts[c, bi, 1] = np.sum(mb_ & (hh == 1))
        cnt_n = np.bincount(dl, minlength=NLpad)
        K = max(K, int(cnt_n.max()))

    cmax = counts.max(axis=0)          # [NB, 2] uniform padded counts
    # windows per half; stream order within half h: blocks 0..NB-1
    W_h = []
    pref = np.zeros((2, NB + 1), np.int64)
    for h in (0, 1):
        pref[h, 1:] = np.cumsum(cmax[:, h])
        W_h.append(int(-(-pref[h, NB] // 128)))
    W_lo, W_hi = W_h
    Wtot = W_lo + W_hi

    # segments (matmul units): per half, per block, windows it spans
    segs = []   # (b, h, w, r0, r1)  rows [r0, r1) within window w (half-local)
    for h in (0, 1):
        for b in range(NB):
            p0, p1 = int(pref[h, b]), int(pref[h, b + 1])
            if p1 == p0:
                continue
            wlo, whi = p0 // 128, (p1 - 1) // 128
            for w in range(wlo, whi + 1):
                r0 = max(0, p0 - w * 128)
                r1 = min(128, p1 - w * 128)
                segs.append((b, h, w, r0, r1))
    S = len(segs)
    real_edges = counts.sum()
    pad_frac = (Wtot * 128 * P - real_edges) / max(real_edges, 1)

    in_maps = []
    for c in range(P):
        s, dl, w, hh, b = per_core[c]
        ewrow = np.zeros((128, NB * K), np.float32)
        fill_n = np.zeros(NLpad, np.int64)
        for e in range(len(dl)):
            n = dl[e]
            ewrow[n % 128, (n // 128) * K + fill_n[n]] = w[e]
            fill_n[n] += 1

        idx = [np.zeros((16, 8 * max(W_lo, 1)), np.int16),
               np.zeros((16, 8 * max(W_hi, 1)), np.int16)]
        dstoff = np.zeros((128, S), np.float32)
        ewc = np.zeros((128, S), np.float32)

        # place this core's edges at padded-stream positions, then re-sort
        # each 128-row window by src table row (HBM locality for the gather;
        # the one-hot M absorbs any within-window permutation)
        ptr = 0
        edge_pos = {}   # (h, stream_pos) -> edge index; only real edges
        for h in (0, 1):
            for bi in range(NB):
                cnt = int(counts[c, bi, h])
                base = int(pref[h, bi])
                for k in range(cnt):
                    edge_pos[(h, base + k)] = ptr + k
                ptr += cnt
        for h in (0, 1):
            for wdx in range(W_h[h]):
                rows = [(h, wdx * 128 + j) for j in range(128)]
                es = [edge_pos.get(r) for r in rows]
                keyed = sorted((e for e in es if e is not None), key=lambda e: s[e])
                npad = sum(1 for e in es if e is None)
                for j, r in enumerate(rows):
                    if j < len(keyed):
                        edge_pos[r] = keyed[j]
                    elif r in edge_pos:
                        del edge_pos[r]
        # idx arrays per window
        for h in (0, 1):
            for wdx in range(W_h[h]):
                for j in range(128):
                    e = edge_pos.get((h, wdx * 128 + j))
                    ii = 0 if e is None else int(s[e] - h * HALF)
                    idx[h][j % 16, wdx * 8 + j // 16] = ii
        # per-seg columns by block membership (rows are permuted within
        # windows, so an edge can sit outside its block's contiguous range)
        seg_of = {(bi, h, wdx): si for si, (bi, h, wdx, r0, r1) in enumerate(segs)}
        for h in (0, 1):
            for wdx in range(W_h[h]):
                for j in range(128):
                    e = edge_pos.get((h, wdx * 128 + j))
                    if e is not None:
                        bb = int(dl[e]) // BLK
                        si = seg_of[(bb, h, wdx)]
                        dstoff[j, si] = float(dl[e] - bb * BLK)
                        ewc[j, si] = w[e]

        xT = np.zeros((128, NLpad), np.float32)
        xT[:, perms[c]] = x[c * NL:(c + 1) * NL].T
        xT = xT.astype(ml_dtypes.bfloat16)
        in_maps.append({
            "xT": xT,
            "idxlo": np.tile(idx[0], (8, 1)),
            "idxhi": np.tile(idx[1], (8, 1)),
            "dstoff": dstoff,
            "ewc": ewc,
            "ewrow": ewrow,
        })

    meta = dict(NL=NL, NB=NB, NLpad=NLpad, K=K, HALF=HALF, LA=LA, LB=LB,
                W_lo=W_lo, W_hi=W_hi, S=S, segs=segs, perms=perms,
                pad_frac=float(pad_frac))
    return in_maps, meta


def host_consts(W0, b0, W1, b1, gamma0, beta0, mean0, var0,
                gamma1, beta1, mean1, var1, act_params):
    eps = 1e-5
    s0 = (gamma0 / np.sqrt(var0 + eps)).astype(np.float32)
    s1 = (gamma1 / np.sqrt(var1 + eps)).astype(np.float32)
    c0 = ((b0 - mean0) * s0 + beta0).astype(np.float32)
    c1 = ((b1 - mean1) * s1 + beta1).astype(np.float32)
    crow = np.concatenate([c0, c1]).reshape(1, 256)
    alpha = float(1.0 / (1.0 + np.exp(-float(np.asarray(act_params).reshape(-1)[0]))))
    ident = np.eye(128, dtype=np.float32)
    iota = np.tile(np.arange(128, dtype=np.float32)[None, :], (128, 1)).astype(ml_dtypes.bfloat16)
    return {
        "w0": (W0 * s0[None, :]).astype(ml_dtypes.bfloat16),
        "w1": (W1 * s1[None, :]).astype(ml_dtypes.bfloat16),
        "crow": crow,
        "ident": ident,
        "iota": iota,
    }, alpha


# ---------------------------------------------------------------- builder

def build(meta, cfg: Cfg):
    NL, NB, NLpad = meta["NL"], meta["NB"], meta["NLpad"]
    K, HALF = meta["K"], meta["HALF"]
    LA, LB = meta["LA"], meta["LB"]
    NBA = LA // 128              # blocks in the low piece
    W_lo, W_hi, S = meta["W_lo"], meta["W_hi"], meta["S"]
    segs = meta["segs"]
    N, P, GS = cfg.N, cfg.P, cfg.GS
    alpha = float(meta["alpha"])
    gelu_fn = AF.Gelu if cfg.gelu_hw else AF.Sigmoid

    nc = bacc.Bacc(None, target_bir_lowering=False, debug=False)

    xT_ext = nc.declare_dram_parameter("xT", [128, NLpad], BF16, isOutput=False)
    idxlo_ext = nc.declare_dram_parameter("idxlo", [128, 8 * max(W_lo, 1)], I16, isOutput=False)
    idxhi_ext = nc.declare_dram_parameter("idxhi", [128, 8 * max(W_hi, 1)], I16, isOutput=False)
    dstoff_ext = nc.declare_dram_parameter("dstoff", [128, S], F32, isOutput=False)
    ewc_ext = nc.declare_dram_parameter("ewc", [128, S], F32, isOutput=False)
    ewrow_ext = nc.declare_dram_parameter("ewrow", [128, NB * K], F32, isOutput=False)
    w0_ext = nc.declare_dram_parameter("w0", [128, 128], BF16, isOutput=False)
    w1_ext = nc.declare_dram_parameter("w1", [128, 128], BF16, isOutput=False)
    crow_ext = nc.declare_dram_parameter("crow", [1, 256], F32, isOutput=False)
    ident_ext = nc.declare_dram_parameter("ident", [128, 128], F32, isOutput=False)
    iota_ext = nc.declare_dram_parameter("iota", [128, 128], BF16, isOutput=False)
    out_ext = nc.declare_dram_parameter("out", [NL, 128], F32, isOutput=True)

    warm_loc = nc.dram_tensor("warm_loc", [1, 128], F32)
    warm_full = nc.dram_tensor("warm_full", [8, 128], F32, addr_space="Shared")
    hs_loca = nc.dram_tensor("hs_loca", [LA, 128], BF16)
    hs_locb = nc.dram_tensor("hs_locb", [LB, 128], BF16)
    hs_full = nc.dram_tensor("hs_full", [N, 128], BF16, addr_space="Shared")
    hs2_loca = nc.dram_tensor("hs2_loca", [LA, 128], BF16)
    hs2_locb = nc.dram_tensor("hs2_locb", [LB, 128], BF16)
    hs2_full = nc.dram_tensor("hs2_full", [N, 128], BF16, addr_space="Shared")

    groups = [list(range(P))]

    with tile.TileContext(nc, num_cores=P) as tc, ExitStack() as ctx:
        nc.gpsimd.load_library(library_config.mlp)
        cst = ctx.enter_context(tc.tile_pool(name="cst", bufs=1))
        w0_sb = cst.tile([128, 128], BF16)
        w1_sb = cst.tile([128, 128], BF16)
        crow_sb = cst.tile([1, 256], F32)
        ident_sb = cst.tile([128, 128], F32)
        xT_region = cst.tile([128, NLpad], BF16)
        iota_sb = cst.tile([128, 128], BF16)
        ones_row = cst.tile([1, 128], F32)
        warm_sb = cst.tile([1, 128], F32)
        idxlo_sb = cst.tile([128, 8 * max(W_lo, 1)], I16)
        idxhi_sb = cst.tile([128, 8 * max(W_hi, 1)], I16)
        dstoff_sb = cst.tile([128, S], F32)
        ewc_sb = cst.tile([128, S], F32)
        ewrow_sb = cst.tile([128, NB * K], F32)
        deg_sb = cst.tile([128, NB], F32)
        dinv_sb = cst.tile([128, NB], F32)
        c0_rep = cst.tile([128, 128], F32)
        c1_rep = cst.tile([128, 128], F32)
        y1_region = cst.tile([128, NB * 128], F32)
        part_region = cst.tile([128, NB * 128], F32)
        hs_region = cst.tile([128, NB * 128], BF16)
        hs2_region = cst.tile([128, NB * 128], BF16)

        nc.sync.dma_start(out=xT_region[:, :], in_=xT_ext[:, :])
        nc.sync.dma_start(out=w0_sb[:, :], in_=w0_ext[:, :])
        nc.sync.dma_start(out=ewrow_sb[:, :], in_=ewrow_ext[:, :])
        nc.sync.dma_start(out=w1_sb[:, :], in_=w1_ext[:, :])
        nc.sync.dma_start(out=crow_sb[:, :], in_=crow_ext[:, :])
        nc.sync.dma_start(out=ident_sb[:, :], in_=ident_ext[:, :])
        nc.sync.dma_start(out=iota_sb[:, :], in_=iota_ext[:, :])
        nc.sync.dma_start(out=idxlo_sb[:, :], in_=idxlo_ext[:, :])
        nc.sync.dma_start(out=idxhi_sb[:, :], in_=idxhi_ext[:, :])
        nc.sync.dma_start(out=dstoff_sb[:, :], in_=dstoff_ext[:, :])
        nc.sync.dma_start(out=ewc_sb[:, :], in_=ewc_ext[:, :])
        nc.vector.memset(ones_row[:, :], 1.0)
        # collectives warmup: first cc call pays ~15-20us extra; hide it here
        nc.vector.memset(warm_sb[:, :], 0.0)
        nc.sync.dma_start(out=warm_loc[:, :], in_=warm_sb[:, :])
        nc.gpsimd.collective_compute(
            "AllGather", OP.bypass, replica_groups=groups,
            ins=[warm_loc[:, :]], outs=[warm_full[:, :]])

        mpool = ctx.enter_context(tc.tile_pool(name="mh", bufs=20))
        gpool = ctx.enter_context(tc.tile_pool(name="gh", bufs=10))
        psm = ctx.enter_context(tc.tile_pool(name="ps", bufs=6, space="PSUM"))
        wk = ctx.enter_context(tc.tile_pool(name="wk", bufs=4))
        psh = ctx.enter_context(tc.tile_pool(name="psh", bufs=1, space="PSUM"))
        pst = ctx.enter_context(tc.tile_pool(name="pst", bufs=1, space="PSUM"))
        sbp = ctx.enter_context(tc.tile_pool(name="sbp", bufs=3))

        def build_m(si, pool):
            m = pool.tile([128, 128], BF16, tag="m")
            nc.vector.tensor_scalar(m[:, :], iota_sb[:, :],
                                    dstoff_sb[:, si:si + 1], ewc_sb[:, si:si + 1],
                                    OP.is_equal, OP.mult)
            return m

        # deg via one reduction of the per-dst-row ew layout; +1 self-loop
        nc.vector.tensor_reduce(
            deg_sb[:, :], ewrow_sb[:, :].rearrange("p (b k) -> p b k", k=K),
            mybir.AxisListType.X, OP.add)
        nc.vector.tensor_scalar_add(deg_sb[:, :], deg_sb[:, :], 1.0)
        nc.scalar.activation(dinv_sb[:, :], deg_sb[:, :], AF.Sqrt)
        nc.vector.tensor_scalar_max(dinv_sb[:, :], dinv_sb[:, :], 0.5)
        nc.vector.reciprocal(dinv_sb[:, :], dinv_sb[:, :])

        for i, rep in ((0, c0_rep), (1, c1_rep)):
            pr = psh.tile([128, 128], F32, tag="h")
            nc.tensor.matmul(pr[:, :], ones_row[:, :], crow_sb[0:1, i * 128:(i + 1) * 128])
            nc.scalar.activation(rep[:, :], pr[:, :], AF.Copy)

        def table_block_l1(t):
            col = slice(t * 128, (t + 1) * 128)
            ph = psh.tile([128, 128], F32, tag="h")
            nc.tensor.matmul(ph[:, :], xT_region[:, col], w0_sb[:, :])
            nc.vector.tensor_scalar(hs_region[:, col], ph[:, :],
                                    dinv_sb[:, t:t + 1], None, OP.mult)

        def table_block_l2(t):
            col = slice(t * 128, (t + 1) * 128)
            ptr_ = pst.tile([128, 128], F32, tag="tr")
            nc.tensor.transpose(ptr_[:, :], y1_region[:, col], ident_sb[:, :])
            lhsT = sbp.tile([128, 128], BF16, tag="lhs")
            nc.vector.tensor_copy(lhsT[:, :], ptr_[:, :])
            ph = psh.tile([128, 128], F32, tag="h")
            nc.tensor.matmul(ph[:, :], lhsT[:, :], w1_sb[:, :])
            nc.vector.tensor_scalar(hs2_region[:, col], ph[:, :],
                                    dinv_sb[:, t:t + 1], None, OP.mult)

        def _store_piece(region, loc, b0, nrows):
            # store region cols [b0*128, b0*128+nrows) -> loc[0:nrows]
            full_nb = nrows // 128
            rem = nrows - full_nb * 128
            c0 = b0 * 128
            if full_nb:
                nc.sync.dma_start(
                    out=loc[0:full_nb * 128, :].rearrange("(b p) f -> p b f", p=128),
                    in_=region[:, c0:c0 + full_nb * 128].rearrange("p (b f) -> p b f", f=128))
            if rem:
                nc.sync.dma_start(
                    out=loc[full_nb * 128:nrows, :],
                    in_=region[0:rem, c0 + full_nb * 128:c0 + (full_nb + 1) * 128])

        # segments grouped by block, in (half, window) order per block
        segs_of_block = [[] for _ in range(NB)]
        for si, (b, h, w, r0, r1) in enumerate(segs):
            segs_of_block[b].append((si, h, w))
        W_half = [W_lo, W_hi]

        def scatter_pass(table_full, post_fn):
            # lo sub-phase needs only table rows [0, HALF) (AG piece a);
            # hi sub-phase needs [HALF, N) (piece b). Per block: lo segs
            # accumulate into part_region, hi segs re-accumulate in PSUM and
            # the post adds both.
            g_tiles = [{}, {}]
            idx_sb = [idxlo_sb, idxhi_sb]
            half_view = [table_full[0:HALF, :], table_full[HALF:N, :]]

            def g_window(h, w):
                ch = w // GS
                if ch not in g_tiles[h]:
                    lo = ch * GS
                    hi = min(W_half[h], lo + GS)
                    Sn = hi - lo
                    t_ = gpool.tile([128, Sn, 128], BF16, tag=f"gt{h}")
                    nc.gpsimd.dma_gather(
                        t_[:, :, :], half_view[h], idx_sb[h][:, lo * 8:hi * 8],
                        num_idxs=Sn * 128, num_idxs_reg=Sn * 128, elem_size=128)
                    g_tiles[h].clear()
                    g_tiles[h][ch] = (t_, lo)
                t_, lo = g_tiles[h][ch]
                return t_[:, w - lo, :]

            for b in range(NB):
                sl = [sg for sg in segs_of_block[b] if sg[1] == 0]
                col = slice(b * 128, (b + 1) * 128)
                if sl:
                    pm = psm.tile([128, 128], F32, tag="pm")
                    for j, (si, h, w) in enumerate(sl):
                        m = build_m(si, mpool)
                        g_ap = g_window(h, w)
                        nc.tensor.matmul(pm[:, :], m[:, :], g_ap,
                                         start=(j == 0), stop=(j == len(sl) - 1))
                    nc.scalar.activation(part_region[:, col], pm[:, :], AF.Copy)
                else:
                    nc.vector.memset(part_region[:, col], 0.0)
            for b in range(NB):
                sl = [sg for sg in segs_of_block[b] if sg[1] == 1]
                col = slice(b * 128, (b + 1) * 128)
                pm = psm.tile([128, 128], F32, tag="pm")
                if sl:
                    for j, (si, h, w) in enumerate(sl):
                        m = build_m(si, mpool)
                        g_ap = g_window(h, w)
                        nc.tensor.matmul(pm[:, :], m[:, :], g_ap,
                                         start=(j == 0), stop=(j == len(sl) - 1))
                    nc.vector.tensor_add(part_region[:, col], part_region[:, col],
                                         pm[:, :])
                post_fn(b, wk)

        # ---------------- layer 1 (piece a first so its AG fires early)
        for t in range(NBA):
            table_block_l1(t)
        _store_piece(hs_region, hs_loca, 0, LA)
        nc.gpsimd.collective_compute(
            "AllGather", OP.bypass, replica_groups=groups,
            ins=[hs_loca[:, :]], outs=[hs_full[0:HALF, :]])
        for t in range(NBA, NB):
            table_block_l1(t)
        _store_piece(hs_region, hs_locb, NBA, LB)
        nc.gpsimd.collective_compute(
            "AllGather", OP.bypass, replica_groups=groups,
            ins=[hs_locb[:, :]], outs=[hs_full[HALF:N, :]])

        def post1(b, wkp):
            col = slice(b * 128, (b + 1) * 128)
            u = wkp.tile([128, 128], F32, tag="u")
            nc.vector.tensor_add(u[:, :], part_region[:, col], hs_region[:, col])
            nc.vector.tensor_scalar(u[:, :], u[:, :], dinv_sb[:, b:b + 1], None, OP.mult)
            nc.vector.tensor_add(u[:, :], u[:, :], c0_rep[:, :])
            r = wkp.tile([128, 128], F32, tag="r")
            g = wkp.tile([128, 128], F32, tag="g")
            nc.scalar.activation(r[:, :], u[:, :], AF.Relu)
            nc.scalar.activation(g[:, :], u[:, :], gelu_fn)
            nc.vector.tensor_scalar(r[:, :], r[:, :], alpha, None, OP.mult)
            nc.vector.tensor_scalar(g[:, :], g[:, :], 1.0 - alpha, None, OP.mult)
            nc.vector.tensor_add(y1_region[:, col], r[:, :], g[:, :])
            table_block_l2(b)
            if b == NBA - 1:
                _store_piece(hs2_region, hs2_loca, 0, LA)
                nc.gpsimd.collective_compute(
                    "AllGather", OP.bypass, replica_groups=groups,
                    ins=[hs2_loca[:, :]], outs=[hs2_full[0:HALF, :]])

        scatter_pass(hs_full, post1)

        # ---------------- layer 2
        _store_piece(hs2_region, hs2_locb, NBA, LB)
        nc.gpsimd.collective_compute(
            "AllGather", OP.bypass, replica_groups=groups,
            ins=[hs2_locb[:, :]], outs=[hs2_full[HALF:N, :]])

        out_region = y1_region

        def post2(b, wkp):
            col = slice(b * 128, (b + 1) * 128)
            u = wkp.tile([128, 128], F32, tag="u")
            nc.vector.tensor_add(u[:, :], part_region[:, col], hs2_region[:, col])
            nc.vector.tensor_scalar(u[:, :], u[:, :], dinv_sb[:, b:b + 1], None, OP.mult)
            nc.vector.tensor_add(out_region[:, col], u[:, :], c1_rep[:, :])
            # stream this block's rows out as soon as they're final
            lo = b * 128
            hi = min(NL, lo + 128)
            nc.sync.dma_start(out=out_ext[lo:hi, :],
                              in_=out_region[0:hi - lo, col])

        scatter_pass(hs2_full, post2)

    nc.finalize()
    return nc


# ---------------------------------------------------------------- runners

def prep_all(inputs, cfg: Cfg):
    in_maps, meta = host_prep(inputs["x"], inputs["edge_index"],
                              inputs["edge_weight"], cfg)
    consts, alpha = host_consts(
        inputs["W0"], inputs["b0"], inputs["W1"], inputs["b1"],
        inputs["gamma0"], inputs["beta0"], inputs["mean0"],
        inputs["var0"], inputs["gamma1"], inputs["beta1"],
        inputs["mean1"], inputs["var1"], inputs["act_params"])
    meta["alpha"] = alpha
    for m in in_maps:
        m.update(consts)
    return in_maps, meta


def unshard(results, cfg: Cfg, meta=None):
    NL = cfg.N // cfg.P
    out = np.zeros((cfg.N, cfg.D), np.float32)
    for c in range(cfg.P):
        r = results[c]["out"]
        if meta is not None and "perms" in meta:
            out[c * NL:(c + 1) * NL] = r[meta["perms"][c]]
        else:
            out[c * NL:(c + 1) * NL] = r
    return out


# ---------------------------------------------------------------- entrypoint

def _install_dge_patch():
    """walrus needs --dge-levels=vector_dynamic_offsets for the indirect
    (DynamicAP) gather DMAs this kernel uses."""
    from concourse import bass_utils as _bu
    if getattr(_bu, "_gcn_dge_patched", False):
        return
    _orig = _bu.run_command

    def _patched(argv, **kwargs):
        if argv and "walrus_driver" in str(argv[0]) and not any(
                str(a).startswith("--dge-levels") for a in argv):
            argv = list(argv) + ["--dge-levels=vector_dynamic_offsets"]
        return _orig(argv, **kwargs)

    _bu.run_command = _patched
    _bu._gcn_dge_patched = True


_CFG = Cfg()


def kernel(**inputs):
    """Full-input entrypoint: shard, run on 8 NeuronCores, gather output."""
    import numpy as _np
    _install_dge_patch()
    inputs = {k: _np.asarray(v) for k, v in inputs.items()}
    in_maps, meta = prep_all(inputs, _CFG)
    nc = build(meta, _CFG)
    res = run_bass_kernel_spmd(nc, in_maps, core_ids=list(range(_CFG.P)))
    return unshard([{k: _np.asarray(v) for k, v in r.items()} for r in res.results],
                   _CFG, meta)
